# revision 28
# baseline (speedup 1.0000x reference)
"""Trainium2 Bass kernel for nn_DenseFlashAttention (GNN message passing).

Receivers are bin-packed into (core, tile, row) slots; each edge is assigned
to the core/tile owning its receiver, so the segment softmax and scatter-add
are local to a core.  Phase A computes per-node projections node-range-sharded
and AllGathers a bf16 table; Phase B gathers per-edge rows from it and does
segment ops as dense matmuls against host-built 0/1 S matrices (bf16
stationaries -> fast weight load).  0.25*Wout is folded into the projection
weights.

When the blend/temperature parameters are zero-initialized (mix_scale =
mix_bias = rtw = 0, as reset_parameters() produces), a specialized program is
built: g == 0.5 collapses the radial/tangential blend so the table carries
only q = rp+tp (+ per-node u,v), halving gather and AllGather bytes, and the
receiver-side scalars (u,v,decay,1/temp) are computed locally per tile with
no collective dependency.  Edge slots are split by table half so gathers of
the first half overlap the second AllGather.  A general-weights fallback
program is kept for any other parameter values.
"""
import numpy as np
import ml_dtypes

import concourse.bacc as bacc
import concourse.mybir as mybir
from concourse import tile
from concourse.bass_utils import run_bass_kernel_spmd

BF = ml_dtypes.bfloat16

N_NODES = 20000
N_EDGES = 200000
F = 64
H = 4
M = 32
NCORES = 8
NPC = N_NODES // NCORES          # 2500 nodes per core (range shard)
NPC_PAD = 2560                    # padded to 20 x 128
TROW_TOT = NPC_PAD * NCORES       # 20480
AGB = 2560                        # AllGather batch rows (single batch)
ROWW_G = 640                      # general table row (bf16): rp|tp|16xf32|pad
ROWW_S = 384                      # special table row (bf16): q(256)|uv(8)|pad

f32 = mybir.dt.float32
bf16 = mybir.dt.bfloat16
i16 = mybir.dt.int16
AF = mybir.ActivationFunctionType
ALU = mybir.AluOpType

TRACE = False          # set by test.py for NTFF profiling
TRACE_KW = {}
LAST_RESULT = {}       # exec_time_ns etc. stashed here when TRACE

_CACHE = {}

# Route Exp and Ln to the one activation table holding both, so the
# per-tile softplus (exp then ln) never reloads activation tables.
_orig_gat = None


def _patched_gat(arch):
    t = _orig_gat(arch)
    for k in t:
        if k != "natural_log_exp_and_others":
            t[k] = t[k] - {AF.Exp, AF.Ln}
    return t


def _install_act_table_patch():
    global _orig_gat
    import concourse.bacc as bacc_mod
    if getattr(bacc_mod, "_act_tbl_patched", False):
        return
    _orig_gat = bacc_mod.get_activation_tables
    bacc_mod.get_activation_tables = _patched_gat
    bacc_mod._act_tbl_patched = True


def _np_softplus(v):
    v = np.asarray(v, np.float64)
    return np.log1p(np.exp(-np.abs(v))) + np.maximum(v, 0)


def _pack_receivers(deg):
    """Bin-pack positive-degree nodes into 8*NT bins (cap 128 rows, C*128
    edges) with worst-fit-decreasing."""
    order = np.argsort(-deg, kind="stable")
    order = order[deg[order] > 0]
    for NT, C in [(20, 10), (21, 10), (22, 11), (25, 13)]:
        nbins = NCORES * NT
        cap_e = C * 128
        bins_e = np.zeros(nbins, np.int64)
        bins_r = np.zeros(nbins, np.int64)
        bins = [[] for _ in range(nbins)]
        ok = True
        for n in order:
            d = int(deg[n])
            cand = np.flatnonzero((bins_r < 128) & (bins_e + d <= cap_e))
            if cand.size == 0:
                ok = False
                break
            b = int(cand[np.argmin(bins_e[cand])])
            bins_e[b] += d
            bins_r[b] += 1
            bins[b].append(int(n))
        if ok:
            return NT, C, bins
    raise RuntimeError("packing failed")


def _trow(n):
    """global node id -> table row index (batched-AllGather layout)."""
    c, r = n // NPC, n % NPC
    return (r // AGB) * (AGB * NCORES) + c * AGB + (r % AGB)


def _wrap16(idx, reps=8):
    """idx [n] int -> [16, n/16] wrapped, replicated to [16*reps, n/16]."""
    n = idx.shape[0]
    assert n % 16 == 0
    w = np.ascontiguousarray(idx.reshape(n // 16, 16).T).astype(np.int16)
    return np.tile(w, (reps, 1))


def _preprocess(x, edge_index, edge_len):
    sender = np.asarray(edge_index[0]).astype(np.int64)
    receiver = np.asarray(edge_index[1]).astype(np.int64)
    el = np.asarray(edge_len, np.float32)
    deg = np.bincount(receiver, minlength=N_NODES)

    NT, C, bins = _pack_receivers(deg)
    EC = NT * C * 128  # edge slots per core

    eorder = np.argsort(receiver, kind="stable")
    starts = np.searchsorted(receiver[eorder], np.arange(N_NODES))
    ends = np.searchsorted(receiver[eorder], np.arange(N_NODES) + 1)

    cores = []
    for c in range(NCORES):
        g1a = np.zeros(EC, np.int64)       # sender table row per slot
        lenv = np.zeros(EC, np.float32)
        own = np.zeros(NT * 128, np.int64)
        s_em = np.zeros((128, EC), np.uint8)
        s_nm = np.zeros((128, EC), np.uint8)
        node_of = np.full(NT * 128, -1, np.int64)
        for t in range(NT):
            b = bins[c * NT + t]
            j = t * C * 128
            for r, n in enumerate(b):
                own[t * 128 + r] = _trow(n)
                node_of[t * 128 + r] = n
                for e in eorder[starts[n]:ends[n]]:
                    g1a[j] = _trow(int(sender[e]))
                    lenv[j] = el[e]
                    blk, p = j // 128, j % 128
                    s_em[p, blk * 128 + r] = 1
                    s_nm[r, blk * 128 + p] = 1
                    j += 1
            assert j <= (t + 1) * C * 128
        wrapped = _wrap16(g1a)
        cores.append(dict(
            g1i=wrapped, g1i_abs=wrapped, owni=_wrap16(own),
            len_pl=np.ascontiguousarray(
                lenv.reshape(NT * C, 128).T).astype(np.float32),
            s_em=s_em.astype(BF), s_nm=s_nm.astype(BF), node_of=node_of,
        ))
    return NT, C, cores


# ====================== specialized program (g = 0.5) ======================

def _build_program_special(NT, C):
    _install_act_table_patch()
    EC = NT * C * 128
    NBLK = NT * C
    G2E = 16 * C          # psS regions
    P1E = G2E
    SCE0 = P1E + 4
    CH0 = SCE0 + 4 * C
    PS_T = CH0 + 8
    nc = bacc.Bacc("TRN2", target_bir_lowering=False, debug=False,
                   num_devices=NCORES)

    xT_d = nc.dram_tensor("xT_in", [65, NPC_PAD], bf16, kind="ExternalInput")
    xpT_d = nc.dram_tensor("xpT_in", [65, NT * 128], bf16, kind="ExternalInput")
    x_perm = nc.dram_tensor("x_perm", [NT * 128, F], f32, kind="ExternalInput")
    g1i_d = nc.dram_tensor("g1i", [128, EC // 16], i16, kind="ExternalInput")
    len_d = nc.dram_tensor("len_pl", [128, NBLK], f32, kind="ExternalInput")
    sem_d = nc.dram_tensor("s_em", [128, EC], bf16, kind="ExternalInput")
    snm_d = nc.dram_tensor("s_nm", [128, EC], bf16, kind="ExternalInput")
    wq_d = nc.dram_tensor("wq", [F, 256], bf16, kind="ExternalInput")
    wuv8_d = nc.dram_tensor("wuv8", [F, 8], bf16, kind="ExternalInput")
    wuvx_d = nc.dram_tensor("wuvx", [65, 16], bf16, kind="ExternalInput")
    dw1_d = nc.dram_tensor("dw1", [F, H * M], bf16, kind="ExternalInput")
    tw1_d = nc.dram_tensor("tw1", [F, H * M], bf16, kind="ExternalInput")
    db1_d = nc.dram_tensor("db1c", [H * M, 1], f32, kind="ExternalInput")
    tb1_d = nc.dram_tensor("tb1c", [H * M, 1], f32, kind="ExternalInput")
    dw2_d = nc.dram_tensor("dw2b", [H * M, 16], bf16, kind="ExternalInput")
    tw2_d = nc.dram_tensor("tw2b", [H * M, 16], bf16, kind="ExternalInput")
    out_d = nc.dram_tensor("out_perm", [NT * 128, F], f32, kind="ExternalOutput")

    scal_d = nc.dram_tensor("scal_d", [NT * 128, 16], f32)
    tbl_sh = nc.dram_tensor("tbl_sh", [AGB, ROWW_S], bf16)
    table = nc.dram_tensor("table", [TROW_TOT, ROWW_S], bf16,
                           addr_space="Shared")

    with tile.TileContext(nc) as tc:
        # ===== Phase A part 1: q|uv table (feeds the AllGathers ASAP) =====
        with (
            tc.tile_pool(name="pa_const", bufs=1) as pc_,
            tc.tile_pool(name="pa", bufs=2) as pa,
            tc.tile_pool(name="pa_ps", bufs=2, space="PSUM") as pap,
        ):
            wq = pc_.tile([F, 256], bf16)
            nc.sync.dma_start(wq[:], wq_d[:])
            wuv8 = pc_.tile([F, 8], bf16)
            nc.sync.dma_start(wuv8[:], wuv8_d[:])
            dw1 = pc_.tile([F, H * M], bf16)
            nc.sync.dma_start(dw1[:], dw1_d[:])
            tw1 = pc_.tile([F, H * M], bf16)
            nc.sync.dma_start(tw1[:], tw1_d[:])
            dw2 = pc_.tile([H * M, 16], bf16)
            nc.sync.dma_start(dw2[:], dw2_d[:])
            tw2 = pc_.tile([H * M, 16], bf16)
            nc.sync.dma_start(tw2[:], tw2_d[:])
            db1 = pc_.tile([H * M, 1], f32)
            nc.sync.dma_start(db1[:], db1_d[:])
            tb1 = pc_.tile([H * M, 1], f32)
            nc.sync.dma_start(tb1[:], tb1_d[:])
            wuvx = pc_.tile([65, 16], bf16)
            nc.sync.dma_start(wuvx[:], wuvx_d[:])

            for b in range(NPC_PAD // 512):
                xT = pa.tile([65, 512], bf16, tag="xT")
                nc.sync.dma_start(xT[:], xT_d[:, b * 512:(b + 1) * 512])
                for it in range(4):
                    r0 = b * 512 + it * 128
                    sl = slice(it * 128, (it + 1) * 128)
                    ps_f = pap.tile([128, 256], f32, tag="ps_f")
                    nc.tensor.matmul(ps_f[:], xT[0:64, sl], wq[:],
                                     start=True, stop=True)
                    ps_u = pap.tile([128, 8], f32, tag="ps_u")
                    nc.tensor.matmul(ps_u[:], xT[0:64, sl], wuv8[:],
                                     start=True, stop=True)
                    rt = pa.tile([128, ROWW_S], bf16, tag="rt")
                    nc.vector.tensor_copy(rt[:, 0:256], ps_f[:])
                    nc.vector.tensor_copy(rt[:, 256:264], ps_u[:])
                    nc.sync.dma_start(tbl_sh[r0:r0 + 128, :], rt[:])

            # ===== Phase A part 2: own-node scalars -> local scal_d =====
            for b in range(NT * 128 // 512):
                xpb = pa.tile([65, 512], bf16, tag="xpb")
                nc.sync.dma_start(xpb[:], xpT_d[:, b * 512:(b + 1) * 512])
                ps_h1 = pap.tile([128, 512], f32, tag="ps_h")
                nc.tensor.matmul(ps_h1[:], dw1[:], xpb[0:64, :],
                                 start=True, stop=True)
                h1 = pa.tile([128, 512], bf16, tag="h1")
                nc.scalar.activation(h1[:], ps_h1[:], AF.Silu, bias=db1[:])
                ps_h2 = pap.tile([128, 512], f32, tag="ps_h")
                nc.tensor.matmul(ps_h2[:], tw1[:], xpb[0:64, :],
                                 start=True, stop=True)
                h2 = pa.tile([128, 512], bf16, tag="h2")
                nc.scalar.activation(h2[:], ps_h2[:], AF.Silu, bias=tb1[:])
                for it in range(4):
                    r0 = b * 512 + it * 128
                    sl = slice(it * 128, (it + 1) * 128)
                    ps_s = pap.tile([128, 16], f32, tag="ps_u")
                    nc.tensor.matmul(ps_s[:], xpb[0:65, sl], wuvx[:],
                                     start=True, stop=False,
                                     skip_group_check=True)
                    nc.tensor.matmul(ps_s[:], h1[:, sl], dw2[:],
                                     start=False, stop=False,
                                     skip_group_check=True)
                    nc.tensor.matmul(ps_s[:], h2[:, sl], tw2[:],
                                     start=False, stop=True,
                                     skip_group_check=True)
                    # irt = 1/(softplus(t)+1e-4) replaces the raw t channel
                    itmp = pa.tile([128, 4], f32, tag="itmp")
                    nc.scalar.activation(itmp[:], ps_s[:, 9:16:2], AF.Exp)
                    nc.scalar.activation(itmp[:], itmp[:], AF.Ln, bias=1.0)
                    nc.vector.tensor_scalar_add(itmp[:], itmp[:], 1e-4)
                    nc.vector.reciprocal(itmp[:], itmp[:])
                    so = pa.tile([128, 16], f32, tag="so")
                    nc.vector.tensor_copy(so[:], ps_s[:])
                    nc.vector.tensor_copy(so[:, 9:16:2], itmp[:])
                    nc.sync.dma_start(scal_d[r0:r0 + 128, :], so[:])

        # ===== AllGather the table =====
        nc.gpsimd.collective_compute(
            "AllGather", ALU.bypass,
            ins=[tbl_sh[:]],
            outs=[table[:]],
            replica_groups=[list(range(NCORES))],
        )

        # ======================= Phase B =======================
        with (
            tc.tile_pool(name="pb_const", bufs=1) as pbc,
            tc.tile_pool(name="pb", bufs=3) as pb,
            tc.tile_pool(name="pbg", bufs=3) as pbg,
            tc.tile_pool(name="pb_val", bufs=2) as pv,
            tc.tile_pool(name="pb_fin", bufs=2) as pf,
            tc.tile_pool(name="ps_main", bufs=2, space="PSUM") as psm,
            tc.tile_pool(name="ps_small", bufs=2, space="PSUM") as pss,
        ):
            g1i = pbc.tile([128, EC // 16], i16)
            nc.sync.dma_start(g1i[:], g1i_d[:])
            len_pl = pbc.tile([128, NBLK], f32)
            nc.sync.dma_start(len_pl[:], len_d[:])
            wq2 = pbc.tile([F, 256], bf16)
            nc.sync.dma_start(wq2[:], wq_d[:])

            for t in range(NT):
                j0 = t * C
                lsl = slice(j0, j0 + C)
                S_t = pb.tile([128, C * 128], bf16, tag="S")
                nc.sync.dma_start(S_t[:], sem_d[:, t * C * 128:(t + 1) * C * 128])
                Sn_t = pb.tile([128, C * 128], bf16, tag="Sn")
                nc.sync.dma_start(Sn_t[:], snm_d[:, t * C * 128:(t + 1) * C * 128])
                g1t = pbg.tile([128, C, ROWW_S], bf16, tag="g1t")
                i0 = t * C * 8
                nc.gpsimd.dma_gather(g1t[:], table[:, 0:ROWW_S],
                                     g1i[:, i0:i0 + C * 8],
                                     C * 128, C * 128, elem_size=ROWW_S,
                                     elem_step=ROWW_S, single_packet=False)
                # own-node q (no gather needed)
                xoT = pb.tile([65, 128], bf16, tag="xoT")
                nc.sync.dma_start(xoT[:], xpT_d[:, t * 128:(t + 1) * 128])
                ps_qo = pss.tile([128, 256], f32, tag="ps_qo")
                nc.tensor.matmul(ps_qo[:], xoT[0:64, :], wq2[:],
                                 start=True, stop=True)
                qob = pf.tile([128, 256], bf16, tag="qob")
                nc.scalar.activation(qob[:], ps_qo[:], AF.Copy)
                # own-node scalars from local DRAM
                gscf = pf.tile([128, 16], f32, tag="gscf")
                nc.sync.dma_start(gscf[:], scal_d[t * 128:(t + 1) * 128, :])
                gscb = pf.tile([128, 16], bf16, tag="gscb")
                nc.vector.tensor_copy(gscb[:], gscf[:])
                psS = pss.tile([128, PS_T], f32, tag="psS")
                for c in range(C):
                    nc.tensor.matmul(psS[:, c * 16:(c + 1) * 16],
                                     Sn_t[:, c * 128:(c + 1) * 128], gscb[:],
                                     start=True, stop=True,
                                     skip_group_check=True)
                g2v = psS[:, 0:G2E].rearrange("p (c k) -> p c k", c=C)
                sUV = g1t[:, :, 256:264]
                dUV = pf.tile([128, C, 8], f32, tag="dUV")
                nc.vector.tensor_tensor(dUV[:], sUV, g2v[:, :, 0:8],
                                        op=ALU.subtract)
                dU = dUV[:].rearrange("p c (h q) -> p c h q", q=2)[:, :, :, 0]
                dV = dUV[:].rearrange("p c (h q) -> p c h q", q=2)[:, :, :, 1]
                d_r = g2v[:, :, 8:16].rearrange("p c (h q) -> p c h q", q=2)[:, :, :, 0]
                irt_e = g2v[:, :, 8:16].rearrange("p c (h q) -> p c h q", q=2)[:, :, :, 1]
                len_bc = len_pl[:, lsl].unsqueeze(2).broadcast_to([128, C, H])
                LRT = pf.tile([128, C, H], f32, tag="LRT")
                TMPa = pf.tile([128, C, H], f32, tag="TMPa")
                nc.vector.tensor_tensor(TMPa[:], d_r, len_bc, op=ALU.mult)
                nc.vector.tensor_tensor(LRT[:], dU, TMPa[:], op=ALU.subtract)
                nc.vector.tensor_tensor(LRT[:], LRT[:], irt_e, op=ALU.mult)
                P1t = pf.tile([128, C, H], bf16, tag="P1t")
                nc.scalar.activation(P1t[:], LRT[:], AF.Exp, scale=0.5)
                APL = pf.tile([128, C, 8], bf16, tag="APL")
                nc.scalar.activation(APL[:, :, 4:8], dV, AF.Exp)
                for c in range(C):
                    nc.tensor.matmul(psS[:, P1E:P1E + 4],
                                     S_t[:, c * 128:(c + 1) * 128],
                                     P1t[:, c, :],
                                     start=(c == 0), stop=(c == C - 1),
                                     skip_group_check=True)
                sct = pf.tile([128, H], f32, tag="sct")
                nc.vector.tensor_scalar_add(sct[:], psS[:, P1E:P1E + 4], 1e-30)
                nc.vector.reciprocal(sct[:], sct[:])
                scv = pf.tile([128, H], bf16, tag="scv")
                nc.vector.tensor_copy(scv[:], sct[:])
                for c in range(C):
                    nc.tensor.matmul(psS[:, SCE0 + c * 4:SCE0 + c * 4 + 4],
                                     Sn_t[:, c * 128:(c + 1) * 128], scv[:],
                                     start=True, stop=True,
                                     skip_group_check=True)
                sce = psS[:, SCE0:SCE0 + 4 * C].rearrange("p (c k) -> p c k", c=C)
                ert = pf.tile([128, C, H], f32, tag="ert")
                nc.vector.tensor_tensor(ert[:], P1t[:], sce, op=ALU.mult)
                nc.vector.tensor_tensor(APL[:, :, 0:4], ert[:], ert[:],
                                        op=ALU.mult)
                edup = pf.tile([128, C, 8, 2], bf16, tag="edup")
                nc.vector.tensor_copy(
                    edup[:], APL[:].unsqueeze(3).broadcast_to([128, C, 8, 2]))
                val = pv.tile([128, C, 512], bf16, tag="val")
                for a in range(2):
                    for h in range(H):
                        o = a * 256 + h * 64
                        nc.vector.tensor_tensor(
                            val[:, :, o:o + 64].rearrange(
                                "p c (f2 q) -> p c f2 q", q=2),
                            g1t[:, :, h * 64:h * 64 + 64].rearrange(
                                "p c (f2 q) -> p c f2 q", q=2),
                            edup[:, :, a * 4 + h, :].unsqueeze(2)
                                .broadcast_to([128, C, 32, 2]),
                            op=ALU.mult)
                ps_main = psm.tile([128, 512], f32, tag="ps_main")
                for c in range(C):
                    sL = S_t[:, c * 128:(c + 1) * 128]
                    nc.tensor.matmul(ps_main[:], sL, val[:, c, :],
                                     start=(c == 0), stop=(c == C - 1))
                    nc.tensor.matmul(psS[:, CH0:CH0 + 8], sL, APL[:, c, :],
                                     start=(c == 0), stop=(c == C - 1),
                                     skip_group_check=True)
                # finalize: m = 0.25*(iDr*P_er + iDt*P_et) - C1*q_own
                #           C1 = 0.25*(Dr/(Dr+e) + Dt/(Dt+e))
                i8 = pf.tile([128, 8], f32, tag="i8")
                nc.vector.tensor_scalar_add(i8[:], psS[:, CH0:CH0 + 8], 1e-9)
                nc.vector.reciprocal(i8[:], i8[:])
                t8 = pf.tile([128, 8], f32, tag="t8")
                nc.vector.tensor_tensor(t8[:], i8[:], psS[:, CH0:CH0 + 8],
                                        op=ALU.mult)
                i12 = pf.tile([128, 12], f32, tag="i12")
                nc.vector.tensor_scalar_mul(i12[:, 0:8], i8[:], 0.25)
                nc.vector.tensor_tensor(i12[:, 8:12], t8[:, 0:4], t8[:, 4:8],
                                        op=ALU.add)
                nc.vector.tensor_scalar_mul(i12[:, 8:12], i12[:, 8:12], 0.25)
                idup = pf.tile([128, 12, 2], bf16, tag="idup")
                nc.vector.tensor_copy(
                    idup[:], i12[:].unsqueeze(2).broadcast_to([128, 12, 2]))
                Pm = pf.tile([128, 512], bf16, tag="Pm")
                nc.scalar.activation(Pm[:], ps_main[:], AF.Copy)
                tA = pf.tile([128, 256], bf16, tag="tA")
                tB = pf.tile([128, 256], bf16, tag="tB")
                nc.vector.tensor_tensor(
                    tA[:].rearrange("p (h f2 q) -> p h f2 q", h=H, q=2),
                    Pm[:, 0:256].rearrange("p (h f2 q) -> p h f2 q", h=H, q=2),
                    idup[:, 0:4, :].unsqueeze(2)
                        .broadcast_to([128, H, 32, 2]),
                    op=ALU.mult)
                nc.vector.tensor_tensor(
                    tB[:].rearrange("p (h f2 q) -> p h f2 q", h=H, q=2),
                    Pm[:, 256:512].rearrange("p (h f2 q) -> p h f2 q", h=H, q=2),
                    idup[:, 4:8, :].unsqueeze(2)
                        .broadcast_to([128, H, 32, 2]),
                    op=ALU.mult)
                nc.vector.tensor_tensor(tA[:], tA[:], tB[:], op=ALU.add)
                nc.vector.tensor_tensor(
                    tB[:].rearrange("p (h f2 q) -> p h f2 q", h=H, q=2),
                    qob[:].rearrange("p (h f2 q) -> p h f2 q", h=H, q=2),
                    idup[:, 8:12, :].unsqueeze(2)
                        .broadcast_to([128, H, 32, 2]),
                    op=ALU.mult)
                nc.vector.tensor_tensor(tA[:], tA[:], tB[:], op=ALU.subtract)
                m1 = pf.tile([128, 128], bf16, tag="m1")
                nc.vector.tensor_tensor(m1[:], tA[:, 0:128], tA[:, 128:256],
                                        op=ALU.add)
                mmb = pf.tile([128, F], bf16, tag="mmb")
                nc.vector.tensor_tensor(mmb[:], m1[:, 0:64], m1[:, 64:128],
                                        op=ALU.add)
                xp = pb.tile([128, F], f32, tag="xp")
                nc.sync.dma_start(xp[:], x_perm[t * 128:(t + 1) * 128, :])
                ob = pf.tile([128, F], f32, tag="ob")
                nc.vector.tensor_tensor(ob[:], mmb[:], xp[:], op=ALU.add)
                nc.sync.dma_start(out_d[t * 128:(t + 1) * 128, :], ob[:])

    nc.compile()
    return nc


# ====================== general program (any weights) ======================

def _build_program_general(NT, C, w):
    _install_act_table_patch()
    EC = NT * C * 128
    NBLK = NT * C
    G2E = 16 * C
    P1E = G2E
    SCE0 = P1E + 4
    CH0 = SCE0 + 4 * C
    PS_T = CH0 + 24
    nc = bacc.Bacc("TRN2", target_bir_lowering=False, debug=False,
                   num_devices=NCORES)

    xT_d = nc.dram_tensor("xT_in", [65, NPC_PAD], bf16, kind="ExternalInput")
    x_perm = nc.dram_tensor("x_perm", [NT * 128, F], f32, kind="ExternalInput")
    g1i_d = nc.dram_tensor("g1i_abs", [128, EC // 16], i16, kind="ExternalInput")
    owni_d = nc.dram_tensor("owni", [128, NT * 8], i16, kind="ExternalInput")
    len_d = nc.dram_tensor("len_pl", [128, NBLK], f32, kind="ExternalInput")
    sem_d = nc.dram_tensor("s_em", [128, EC], bf16, kind="ExternalInput")
    snm_d = nc.dram_tensor("s_nm", [128, EC], bf16, kind="ExternalInput")
    wrwt_d = nc.dram_tensor("wrwt", [F, 512], bf16, kind="ExternalInput")
    wuv_d = nc.dram_tensor("wuv_ext", [65, 16], bf16, kind="ExternalInput")
    dw1_d = nc.dram_tensor("dw1", [F, H * M], bf16, kind="ExternalInput")
    tw1_d = nc.dram_tensor("tw1", [F, H * M], bf16, kind="ExternalInput")
    db1_d = nc.dram_tensor("db1c", [H * M, 1], f32, kind="ExternalInput")
    tb1_d = nc.dram_tensor("tb1c", [H * M, 1], f32, kind="ExternalInput")
    dw2_d = nc.dram_tensor("dw2b", [H * M, 16], bf16, kind="ExternalInput")
    tw2_d = nc.dram_tensor("tw2b", [H * M, 16], bf16, kind="ExternalInput")
    rtw_d = nc.dram_tensor("rtw_sb", [128, H], f32, kind="ExternalInput")
    out_d = nc.dram_tensor("out_perm", [NT * 128, F], f32, kind="ExternalOutput")

    table_shs = [nc.dram_tensor(f"table_sh{b}", [AGB, ROWW_G], bf16)
                 for b in range(NPC_PAD // AGB)]
    table = nc.dram_tensor("table", [TROW_TOT, ROWW_G], bf16, addr_space="Shared")

    ms = w["mix_scale"]
    mb = w["mix_bias"]

    with tile.TileContext(nc) as tc:
        with (
            tc.tile_pool(name="pa_const", bufs=1) as pc_,
            tc.tile_pool(name="pa", bufs=2) as pa,
            tc.tile_pool(name="pa_ps", bufs=2, space="PSUM") as pap,
        ):
            wrwt = pc_.tile([F, 512], bf16)
            nc.sync.dma_start(wrwt[:], wrwt_d[:])
            wuv = pc_.tile([65, 16], bf16)
            nc.sync.dma_start(wuv[:], wuv_d[:])
            dw1 = pc_.tile([F, H * M], bf16)
            nc.sync.dma_start(dw1[:], dw1_d[:])
            tw1 = pc_.tile([F, H * M], bf16)
            nc.sync.dma_start(tw1[:], tw1_d[:])
            dw2 = pc_.tile([H * M, 16], bf16)
            nc.sync.dma_start(dw2[:], dw2_d[:])
            tw2 = pc_.tile([H * M, 16], bf16)
            nc.sync.dma_start(tw2[:], tw2_d[:])
            db1 = pc_.tile([H * M, 1], f32)
            nc.sync.dma_start(db1[:], db1_d[:])
            tb1 = pc_.tile([H * M, 1], f32)
            nc.sync.dma_start(tb1[:], tb1_d[:])

            for b in range(NPC_PAD // 512):
                xT = pa.tile([65, 512], bf16, tag="xT")
                nc.sync.dma_start(xT[:], xT_d[:, b * 512:(b + 1) * 512])
                ps_h1 = pap.tile([128, 512], f32, tag="ps_h")
                nc.tensor.matmul(ps_h1[:], dw1[:], xT[0:64, :],
                                 start=True, stop=True)
                h1 = pa.tile([128, 512], bf16, tag="h1")
                nc.scalar.activation(h1[:], ps_h1[:], AF.Silu, bias=db1[:])
                ps_h2 = pap.tile([128, 512], f32, tag="ps_h")
                nc.tensor.matmul(ps_h2[:], tw1[:], xT[0:64, :],
                                 start=True, stop=True)
                h2 = pa.tile([128, 512], bf16, tag="h2")
                nc.scalar.activation(h2[:], ps_h2[:], AF.Silu, bias=tb1[:])
                for it in range(4):
                    r0 = b * 512 + it * 128
                    sl = slice(it * 128, (it + 1) * 128)
                    ps_f = pap.tile([128, 512], f32, tag="ps_f")
                    nc.tensor.matmul(ps_f[:], xT[0:64, sl], wrwt[:],
                                     start=True, stop=True)
                    ps_s = pap.tile([128, 16], f32, tag="ps_s")
                    nc.tensor.matmul(ps_s[:], xT[0:65, sl], wuv[:],
                                     start=True, stop=False,
                                     skip_group_check=True)
                    nc.tensor.matmul(ps_s[:], h1[:, sl], dw2[:],
                                     start=False, stop=False,
                                     skip_group_check=True)
                    nc.tensor.matmul(ps_s[:], h2[:, sl], tw2[:],
                                     start=False, stop=True,
                                     skip_group_check=True)
                    rt = pa.tile([128, ROWW_G], bf16, tag="rt")
                    nc.vector.tensor_copy(rt[:, 0:512], ps_f[:])
                    nc.vector.tensor_copy(rt[:, 512:544].bitcast(f32), ps_s[:])
                    nc.sync.dma_start(
                        table_shs[r0 // AGB][r0 % AGB:r0 % AGB + 128, :], rt[:])

        for b in range(NPC_PAD // AGB):
            nc.gpsimd.collective_compute(
                "AllGather", ALU.bypass,
                ins=[table_shs[b][:]],
                outs=[table[b * AGB * NCORES:(b + 1) * AGB * NCORES, :]],
                replica_groups=[list(range(NCORES))],
            )

        with (
            tc.tile_pool(name="pb_const", bufs=1) as pbc,
            tc.tile_pool(name="pb_planes", bufs=1) as ppl,
            tc.tile_pool(name="pb", bufs=3) as pb,
            tc.tile_pool(name="pbg", bufs=3) as pbg,
            tc.tile_pool(name="pb_val", bufs=2) as pv,
            tc.tile_pool(name="pb_fin", bufs=2) as pf,
            tc.tile_pool(name="ps_main", bufs=2, space="PSUM") as psm,
            tc.tile_pool(name="ps_small", bufs=2, space="PSUM") as pss,
        ):
            g1i = pbc.tile([128, EC // 16], i16)
            nc.sync.dma_start(g1i[:], g1i_d[:])
            owni = pbc.tile([128, NT * 8], i16)
            nc.sync.dma_start(owni[:], owni_d[:])
            len_pl = pbc.tile([128, NBLK], f32)
            nc.sync.dma_start(len_pl[:], len_d[:])
            rtw_sb = pbc.tile([128, H], f32)
            nc.sync.dma_start(rtw_sb[:], rtw_d[:])

            LEN4 = ppl.tile([128, NBLK, H], f32)
            Q1 = ppl.tile([128, NBLK, H], f32)
            Q2 = ppl.tile([128, NBLK, H], f32)
            Q3 = ppl.tile([128, NBLK, H], f32)
            len_b = len_pl[:].unsqueeze(2).broadcast_to([128, NBLK, H])
            rtw_b = rtw_sb[:].unsqueeze(1).broadcast_to([128, NBLK, H])
            nc.vector.tensor_tensor(LEN4[:], len_b, rtw_b, op=ALU.mult)
            G = ppl.tile([128, NBLK, H], f32)
            for h in range(H):
                nc.scalar.activation(G[:, :, h:h + 1],
                                     len_pl[:].unsqueeze(2), AF.Sigmoid,
                                     scale=float(ms[h]), bias=float(mb[h]))
            OMG = ppl.tile([128, NBLK, H], f32)
            nc.scalar.activation(OMG[:], G[:], AF.Copy, scale=-1.0, bias=1.0)
            nc.vector.tensor_tensor(Q1[:], G[:], G[:], op=ALU.mult)
            nc.vector.tensor_tensor(Q2[:], G[:], OMG[:], op=ALU.mult)
            nc.vector.tensor_tensor(Q3[:], OMG[:], OMG[:], op=ALU.mult)

            for t in range(NT):
                j0 = t * C
                lsl = slice(j0, j0 + C)
                S_t = pb.tile([128, C * 128], bf16, tag="S")
                nc.sync.dma_start(S_t[:], sem_d[:, t * C * 128:(t + 1) * C * 128])
                Sn_t = pb.tile([128, C * 128], bf16, tag="Sn")
                nc.sync.dma_start(Sn_t[:], snm_d[:, t * C * 128:(t + 1) * C * 128])
                g1t = pbg.tile([128, C, ROWW_G], bf16, tag="g1t")
                nc.gpsimd.dma_gather(g1t[:], table[:, 0:ROWW_G],
                                     g1i[:, t * C * 8:(t + 1) * C * 8],
                                     C * 128, C * 128, elem_size=ROWW_G,
                                     elem_step=ROWW_G, single_packet=False)
                gown = pbg.tile([128, 1, ROWW_G], bf16, tag="gown")
                nc.gpsimd.dma_gather(gown[:], table[:, 0:ROWW_G],
                                     owni[:, t * 8:(t + 1) * 8],
                                     128, 128, elem_size=ROWW_G,
                                     elem_step=ROWW_G, single_packet=False)
                psS = pss.tile([128, PS_T], f32, tag="psS")
                gsc = pf.tile([128, 16], bf16, tag="gsc")
                nc.vector.tensor_copy(gsc[:], gown[:, 0, 512:544].bitcast(f32))
                for c in range(C):
                    nc.tensor.matmul(psS[:, c * 16:(c + 1) * 16],
                                     Sn_t[:, c * 128:(c + 1) * 128], gsc[:],
                                     start=True, stop=True,
                                     skip_group_check=True)
                g2v = psS[:, 0:G2E].rearrange("p (c k) -> p c k", c=C)
                sUV = g1t[:, :, 512:544].bitcast(f32)
                dUV = pf.tile([128, C, 8], f32, tag="dUV")
                nc.vector.tensor_tensor(dUV[:], sUV[:, :, 0:8], g2v[:, :, 0:8],
                                        op=ALU.subtract)
                dU = dUV[:].rearrange("p c (h q) -> p c h q", q=2)[:, :, :, 0]
                dV = dUV[:].rearrange("p c (h q) -> p c h q", q=2)[:, :, :, 1]
                d_r = g2v[:, :, 8:16].rearrange("p c (h q) -> p c h q", q=2)[:, :, :, 0]
                t_r = g2v[:, :, 8:16].rearrange("p c (h q) -> p c h q", q=2)[:, :, :, 1]
                len_bc = len_pl[:, lsl].unsqueeze(2).broadcast_to([128, C, H])
                LRT = pf.tile([128, C, H], f32, tag="LRT")
                TMPa = pf.tile([128, C, H], f32, tag="TMPa")
                nc.vector.tensor_tensor(TMPa[:], d_r, len_bc, op=ALU.mult)
                nc.vector.tensor_tensor(LRT[:], dU, TMPa[:], op=ALU.subtract)
                nc.vector.tensor_tensor(TMPa[:], LEN4[:, lsl, :], t_r, op=ALU.add)
                nc.scalar.activation(TMPa[:], TMPa[:], AF.Exp)
                nc.scalar.activation(TMPa[:], TMPa[:], AF.Ln, bias=1.0)
                nc.vector.tensor_scalar_add(TMPa[:], TMPa[:], 1e-4)
                nc.vector.reciprocal(TMPa[:], TMPa[:])
                nc.vector.tensor_tensor(LRT[:], LRT[:], TMPa[:], op=ALU.mult)
                P1t = pf.tile([128, C, H], bf16, tag="P1t")
                nc.scalar.activation(P1t[:], LRT[:], AF.Exp, scale=0.5)
                APL = pf.tile([128, C, 24], bf16, tag="APL")
                nc.scalar.activation(APL[:, :, 20:24], dV, AF.Exp)
                for c in range(C):
                    nc.tensor.matmul(psS[:, P1E:P1E + 4],
                                     S_t[:, c * 128:(c + 1) * 128],
                                     P1t[:, c, :],
                                     start=(c == 0), stop=(c == C - 1),
                                     skip_group_check=True)
                scv = pf.tile([128, H], bf16, tag="scv")
                sct = pf.tile([128, H], f32, tag="sct")
                nc.vector.tensor_scalar_add(sct[:], psS[:, P1E:P1E + 4], 1e-30)
                nc.vector.reciprocal(sct[:], sct[:])
                nc.vector.tensor_copy(scv[:], sct[:])
                for c in range(C):
                    nc.tensor.matmul(psS[:, SCE0 + c * 4:SCE0 + c * 4 + 4],
                                     Sn_t[:, c * 128:(c + 1) * 128], scv[:],
                                     start=True, stop=True,
                                     skip_group_check=True)
                sce = psS[:, SCE0:SCE0 + 4 * C].rearrange("p (c k) -> p c k", c=C)
                ert = pf.tile([128, C, H], f32, tag="ert")
                nc.vector.tensor_tensor(ert[:], P1t[:], sce, op=ALU.mult)
                nc.vector.tensor_tensor(APL[:, :, 16:20], ert[:], ert[:],
                                        op=ALU.mult)
                nc.vector.tensor_tensor(APL[:, :, 0:4], Q1[:, lsl, :],
                                        APL[:, :, 16:20], op=ALU.mult)
                nc.vector.tensor_tensor(APL[:, :, 4:8], Q2[:, lsl, :],
                                        APL[:, :, 20:24], op=ALU.mult)
                nc.vector.tensor_tensor(APL[:, :, 8:12], Q2[:, lsl, :],
                                        APL[:, :, 16:20], op=ALU.mult)
                nc.vector.tensor_tensor(APL[:, :, 12:16], Q3[:, lsl, :],
                                        APL[:, :, 20:24], op=ALU.mult)
                wdup = pf.tile([128, C, 16, 2], bf16, tag="wdup")
                nc.vector.tensor_copy(
                    wdup[:], APL[:, :, 0:16].unsqueeze(3)
                        .broadcast_to([128, C, 16, 2]))
                val1 = pv.tile([128, C, 512], bf16, tag="val1")
                val2 = pv.tile([128, C, 512], bf16, tag="val2")
                for (vt, fs0, wi) in ((val1, 0, 0), (val1, 256, 4),
                                      (val2, 0, 8), (val2, 256, 12)):
                    src0 = 0 if wi < 8 else 256
                    for h in range(H):
                        nc.vector.tensor_tensor(
                            vt[:, :, fs0 + h * 64:fs0 + h * 64 + 64].rearrange(
                                "p c (f2 q) -> p c f2 q", q=2),
                            g1t[:, :, src0 + h * 64:src0 + h * 64 + 64].rearrange(
                                "p c (f2 q) -> p c f2 q", q=2),
                            wdup[:, :, wi + h, :].unsqueeze(2)
                                .broadcast_to([128, C, 32, 2]),
                            op=ALU.mult)
                ps_main = psm.tile([128, 512], f32, tag="ps_main")
                for c in range(C):
                    sL = S_t[:, c * 128:(c + 1) * 128]
                    nc.tensor.matmul(ps_main[:], sL, val1[:, c, :],
                                     start=(c == 0), stop=False)
                    nc.tensor.matmul(ps_main[:], sL, val2[:, c, :],
                                     start=False, stop=(c == C - 1))
                    nc.tensor.matmul(psS[:, CH0:CH0 + 24], sL, APL[:, c, :],
                                     start=(c == 0), stop=(c == C - 1),
                                     skip_group_check=True)
                ch = psS[:, CH0:CH0 + 24]
                idrc = pf.tile([128, 16], f32, tag="idrc")
                nc.vector.tensor_scalar_add(idrc[:, 0:8], ch[:, 16:24], 1e-9)
                nc.vector.reciprocal(idrc[:, 0:8], idrc[:, 0:8])
                tmp16 = pf.tile([128, 2, 8], f32, tag="tmp16")
                nc.vector.tensor_tensor(
                    tmp16[:],
                    idrc[:, 0:8].unsqueeze(1).broadcast_to([128, 2, 8]),
                    ch[:, 0:16].rearrange("p (a k) -> p a k", a=2),
                    op=ALU.mult)
                t16 = tmp16[:].rearrange("p a k -> p (a k)")
                nc.vector.tensor_tensor(
                    idrc[:, 8:16].rearrange("p (a h) -> p a h", a=2),
                    t16.rearrange("p (a g h) -> p a g h", a=2, g=2)[:, :, 0, :],
                    t16.rearrange("p (a g h) -> p a g h", a=2, g=2)[:, :, 1, :],
                    op=ALU.add)
                idup = pf.tile([128, 16, 2], bf16, tag="idup")
                nc.vector.tensor_copy(
                    idup[:], idrc[:].unsqueeze(2).broadcast_to([128, 16, 2]))
                Pm = pf.tile([128, 512], bf16, tag="Pm")
                nc.scalar.activation(Pm[:], ps_main[:], AF.Copy)
                t1 = pf.tile([128, 512], bf16, tag="t1")
                t2 = pf.tile([128, 512], bf16, tag="t2")
                for a in range(2):
                    asl = slice(a * 256, (a + 1) * 256)
                    nc.vector.tensor_tensor(
                        t1[:, asl].rearrange("p (h f2 q) -> p h f2 q",
                                             h=H, q=2),
                        Pm[:, asl].rearrange("p (h f2 q) -> p h f2 q",
                                             h=H, q=2),
                        idup[:, a * 4:a * 4 + 4, :].unsqueeze(2)
                            .broadcast_to([128, H, 32, 2]),
                        op=ALU.mult)
                    nc.vector.tensor_tensor(
                        t2[:, asl].rearrange("p (h f2 q) -> p h f2 q",
                                             h=H, q=2),
                        gown[:, 0, asl].rearrange("p (h f2 q) -> p h f2 q",
                                                  h=H, q=2),
                        idup[:, 8 + a * 4:8 + a * 4 + 4, :].unsqueeze(2)
                            .broadcast_to([128, H, 32, 2]),
                        op=ALU.mult)
                nc.vector.tensor_tensor(t1[:], t1[:], t2[:], op=ALU.subtract)
                m2 = pf.tile([128, 256], bf16, tag="m2")
                nc.vector.tensor_tensor(m2[:], t1[:, 0:256], t1[:, 256:512],
                                        op=ALU.add)
                m1 = pf.tile([128, 128], bf16, tag="m1")
                nc.vector.tensor_tensor(m1[:], m2[:, 0:128], m2[:, 128:256],
                                        op=ALU.add)
                mmb = pf.tile([128, F], bf16, tag="mmb")
                nc.vector.tensor_tensor(mmb[:], m1[:, 0:64], m1[:, 64:128],
                                        op=ALU.add)
                xp = pb.tile([128, F], f32, tag="xp")
                nc.sync.dma_start(xp[:], x_perm[t * 128:(t + 1) * 128, :])
                ob = pf.tile([128, F], f32, tag="ob")
                nc.vector.tensor_tensor(ob[:], mmb[:], xp[:], op=ALU.add)
                nc.sync.dma_start(out_d[t * 128:(t + 1) * 128, :], ob[:])

    nc.compile()
    return nc


def kernel(**inputs):
    x = np.asarray(inputs["x"], np.float32)
    edge_index = np.asarray(inputs["edge_index"])
    edge_len = np.asarray(inputs["edge_len"], np.float32)

    NT, C, cores = _preprocess(x, edge_index, edge_len)

    Wp = np.asarray(inputs["Wp"], np.float32)
    Wr = np.asarray(inputs["Wr"], np.float32)
    Wt = np.asarray(inputs["Wt"], np.float32)
    Wout = np.asarray(inputs["Wout"], np.float32)
    ms = np.asarray(inputs["mix_scale"], np.float32)
    mb = np.asarray(inputs["mix_bias"], np.float32)
    rtw = np.asarray(inputs["rtw"], np.float32)
    dconst = (np.asarray(inputs["decay_b2"], np.float64)
              + _np_softplus(inputs["rdls"])).astype(np.float32)
    tconst = (np.asarray(inputs["temp_b2"], np.float64)
              + np.asarray(inputs["rtb"], np.float64)).astype(np.float32)

    special = bool(np.all(ms == 0) and np.all(mb == 0) and np.all(rtw == 0))

    rs = np.asarray(inputs["radial_score"], np.float32)
    ts_ = np.asarray(inputs["tangential_score"], np.float32)
    wu = np.einsum("hfg,hg->fh", Wp, rs)                  # [F, H]
    wv = np.einsum("hfg,hg->fh", Wp, ts_)
    wout_f = 0.25 * Wout
    wd1f = np.einsum("hfg,hgm->fhm", Wp, np.asarray(inputs["decay_W1"], np.float32))
    wt1f = np.einsum("hfg,hgm->fhm", Wp, np.asarray(inputs["temp_W1"], np.float32))

    wuv_ext = np.zeros((65, 16), np.float32)
    wuv_ext[0:64, 0:8:2] = wu
    wuv_ext[0:64, 1:8:2] = wv
    wuv_ext[64, 8:16:2] = dconst
    wuv_ext[64, 9:16:2] = tconst
    dw2b = np.zeros((H * M, 16), np.float32)
    tw2b = np.zeros((H * M, 16), np.float32)
    d_w2 = np.asarray(inputs["decay_w2"], np.float32)
    t_w2 = np.asarray(inputs["temp_w2"], np.float32)
    for h in range(H):
        dw2b[h * M:(h + 1) * M, 8 + 2 * h] = d_w2[h]
        tw2b[h * M:(h + 1) * M, 9 + 2 * h] = t_w2[h]

    shared = {
        "dw1": np.ascontiguousarray(wd1f.reshape(F, H * M)).astype(BF),
        "tw1": np.ascontiguousarray(wt1f.reshape(F, H * M)).astype(BF),
        "db1c": np.ascontiguousarray(
            np.asarray(inputs["decay_b1"], np.float32).reshape(H * M, 1)),
        "tb1c": np.ascontiguousarray(
            np.asarray(inputs["temp_b1"], np.float32).reshape(H * M, 1)),
        "dw2b": dw2b.astype(BF),
        "tw2b": tw2b.astype(BF),
    }
    if special:
        wqF = np.einsum("hfg,gk->hfk", Wr + Wt, wout_f)   # [H, F, F]
        shared["wq"] = np.ascontiguousarray(
            wqF.transpose(1, 0, 2).reshape(F, H * F)).astype(BF)
        wuv8 = np.zeros((F, 8), np.float32)
        wuv8[:, 0:8:2] = wu
        wuv8[:, 1:8:2] = wv
        shared["wuv8"] = wuv8.astype(BF)
        shared["wuvx"] = wuv_ext.astype(BF)
        key = ("v3s", NT, C)
        if key not in _CACHE:
            _CACHE[key] = _build_program_special(NT, C)
    else:
        wrF = np.einsum("hfg,gk->hfk", Wr, wout_f)
        wtF = np.einsum("hfg,gk->hfk", Wt, wout_f)
        shared["wrwt"] = np.ascontiguousarray(np.concatenate(
            [wrF.transpose(1, 0, 2).reshape(F, H * F),
             wtF.transpose(1, 0, 2).reshape(F, H * F)], axis=1)).astype(BF)
        shared["wuv_ext"] = wuv_ext.astype(BF)
        shared["rtw_sb"] = np.tile(rtw[None, :], (128, 1))
        key = ("v3g", NT, C, ms.tobytes(), mb.tobytes())
        if key not in _CACHE:
            _CACHE[key] = _build_program_general(
                NT, C, dict(mix_scale=ms, mix_bias=mb))
    nc = _CACHE[key]

    in_maps = []
    for c in range(NCORES):
        cc = cores[c]
        xr = np.zeros((65, NPC_PAD), np.float32)
        xr[0:64, :NPC] = x[c * NPC:(c + 1) * NPC].T
        xr[64, :] = 1.0
        xp = np.zeros((NT * 128, F), np.float32)
        valid = cc["node_of"] >= 0
        xp[valid] = x[cc["node_of"][valid]]
        im = dict(shared, xT_in=xr.astype(BF), x_perm=xp,
                  len_pl=cc["len_pl"], s_em=cc["s_em"], s_nm=cc["s_nm"])
        if special:
            xpt = np.zeros((65, NT * 128), np.float32)
            xpt[0:64, :] = xp.T
            xpt[64, :] = 1.0
            im["xpT_in"] = xpt.astype(BF)
            im["g1i"] = cc["g1i"]
        else:
            im["g1i_abs"] = cc["g1i_abs"]
            im["owni"] = cc["owni"]
        in_maps.append(im)

    r = run_bass_kernel_spmd(nc, in_maps, list(range(NCORES)),
                             trace=TRACE, **TRACE_KW)
    if TRACE:
        LAST_RESULT["exec_time_ns"] = r.exec_time_ns
        LAST_RESULT["mean_exec_time_ns"] = r.mean_exec_time_ns
        LAST_RESULT["raw"] = r

    out = np.array(x, np.float32, copy=True)  # zero-degree nodes: out = x
    for c in range(NCORES):
        cc = cores[c]
        rows = r.results[c]["out_perm"]
        valid = cc["node_of"] >= 0
        out[cc["node_of"][valid]] = rows[valid]
    return out


# revision 29
# speedup vs baseline: 1.0581x; 1.0581x over previous
"""Trainium2 Bass kernel for nn_DenseFlashAttention (GNN message passing).

Receivers are bin-packed into (core, tile, row) slots; each edge is assigned
to the core/tile owning its receiver, so the segment softmax and scatter-add
are local to a core.  Phase A computes per-node projections node-range-sharded
and AllGathers a bf16 table; Phase B gathers per-edge rows from it and does
segment ops as dense matmuls against host-built 0/1 S matrices (bf16
stationaries -> fast weight load).  0.25*Wout is folded into the projection
weights.

When the blend/temperature parameters are zero-initialized (mix_scale =
mix_bias = rtw = 0, as reset_parameters() produces), a specialized program is
built: g == 0.5 collapses the radial/tangential blend so the table carries
only q = rp+tp (+ per-node u,v), halving gather and AllGather bytes, and the
receiver-side scalars (u,v,decay,1/temp) are computed locally per tile with
no collective dependency.  Edge slots are split by table half so gathers of
the first half overlap the second AllGather.  A general-weights fallback
program is kept for any other parameter values.
"""
import numpy as np
import ml_dtypes

import concourse.bacc as bacc
import concourse.mybir as mybir
from concourse import tile
from concourse.bass_utils import run_bass_kernel_spmd

BF = ml_dtypes.bfloat16

N_NODES = 20000
N_EDGES = 200000
F = 64
H = 4
M = 32
NCORES = 8
NPC = N_NODES // NCORES          # 2500 nodes per core (range shard)
NPC_PAD = 2560                    # padded to 20 x 128
TROW_TOT = NPC_PAD * NCORES       # 20480
HALF = TROW_TOT // 2              # 10240
AGB = 1280                        # AllGather batch rows (2 batches)
ROWW_G = 640                      # general table row (bf16): rp|tp|16xf32|pad
ROWW_S = 384                      # special table row (bf16): q(256)|uv(8)|pad

f32 = mybir.dt.float32
bf16 = mybir.dt.bfloat16
i16 = mybir.dt.int16
AF = mybir.ActivationFunctionType
ALU = mybir.AluOpType

TRACE = False          # set by test.py for NTFF profiling
TRACE_KW = {}
LAST_RESULT = {}       # exec_time_ns etc. stashed here when TRACE

_CACHE = {}

# Route Exp and Ln to the one activation table holding both, so the
# per-tile softplus (exp then ln) never reloads activation tables.
_orig_gat = None


def _patched_gat(arch):
    t = _orig_gat(arch)
    for k in t:
        if k != "natural_log_exp_and_others":
            t[k] = t[k] - {AF.Exp, AF.Ln}
    return t


def _install_act_table_patch():
    global _orig_gat
    import concourse.bacc as bacc_mod
    if getattr(bacc_mod, "_act_tbl_patched", False):
        return
    _orig_gat = bacc_mod.get_activation_tables
    bacc_mod.get_activation_tables = _patched_gat
    bacc_mod._act_tbl_patched = True


def _np_softplus(v):
    v = np.asarray(v, np.float64)
    return np.log1p(np.exp(-np.abs(v))) + np.maximum(v, 0)


def _pack_receivers(deg0, deg1):
    """Bin-pack positive-degree nodes into 8*NT bins: <=128 rows per bin and
    per-table-half edge loads <= (C/2)*128 each (slots are split by sender
    table half at a fixed block boundary)."""
    deg = deg0 + deg1
    order = np.argsort(-deg, kind="stable")
    order = order[deg[order] > 0]
    for NT, C in [(20, 12), (21, 12), (22, 12), (25, 14)]:
        cap_h = (C // 2) * 128
        nbins = NCORES * NT
        b_e0 = np.zeros(nbins, np.int64)
        b_e1 = np.zeros(nbins, np.int64)
        bins_r = np.zeros(nbins, np.int64)
        bins = [[] for _ in range(nbins)]
        ok = True
        for n in order:
            cand = np.flatnonzero((bins_r < 128)
                                  & (b_e0 + deg0[n] <= cap_h)
                                  & (b_e1 + deg1[n] <= cap_h))
            if cand.size == 0:
                ok = False
                break
            b = int(cand[np.argmin(b_e0[cand] + b_e1[cand])])
            b_e0[b] += deg0[n]
            b_e1[b] += deg1[n]
            bins_r[b] += 1
            bins[b].append(int(n))
        if ok:
            return NT, C, bins
    raise RuntimeError("packing failed")


def _trow(n):
    """global node id -> table row index (batched-AllGather layout)."""
    c, r = n // NPC, n % NPC
    return (r // AGB) * (AGB * NCORES) + c * AGB + (r % AGB)


def _wrap16(idx, reps=8):
    """idx [n] int -> [16, n/16] wrapped, replicated to [16*reps, n/16]."""
    n = idx.shape[0]
    assert n % 16 == 0
    w = np.ascontiguousarray(idx.reshape(n // 16, 16).T).astype(np.int16)
    return np.tile(w, (reps, 1))


def _preprocess(x, edge_index, edge_len):
    sender = np.asarray(edge_index[0]).astype(np.int64)
    receiver = np.asarray(edge_index[1]).astype(np.int64)
    el = np.asarray(edge_len, np.float32)
    tr_all = ((sender % NPC // AGB) * (AGB * NCORES)
              + (sender // NPC) * AGB + (sender % NPC) % AGB)
    in0 = tr_all < HALF
    deg0 = np.bincount(receiver[in0], minlength=N_NODES)
    deg1 = np.bincount(receiver[~in0], minlength=N_NODES)

    NT, C, bins = _pack_receivers(deg0, deg1)
    B0 = C // 2
    EC = NT * C * 128  # edge slots per core

    eorder = np.argsort(receiver, kind="stable")
    starts = np.searchsorted(receiver[eorder], np.arange(N_NODES))
    ends = np.searchsorted(receiver[eorder], np.arange(N_NODES) + 1)

    cores = []
    for c in range(NCORES):
        g1r = np.zeros(EC, np.int64)       # split-relative sender row
        g1a = np.zeros(EC, np.int64)       # absolute sender row (general)
        lenv = np.zeros(EC, np.float32)
        own = np.zeros(NT * 128, np.int64)
        s_em = np.zeros((128, EC), np.uint8)
        s_nm = np.zeros((128, EC), np.uint8)
        node_of = np.full(NT * 128, -1, np.int64)
        for t in range(NT):
            b = bins[c * NT + t]
            base = t * C * 128
            e0, e1 = [], []
            for r, n in enumerate(b):
                own[t * 128 + r] = _trow(n)
                node_of[t * 128 + r] = n
                for e in eorder[starts[n]:ends[n]]:
                    tr = int(tr_all[e])
                    (e0 if tr < HALF else e1).append((tr, e, r))

            assert len(e0) <= B0 * 128 and len(e1) <= (C - B0) * 128
            for part, off in ((e0, 0), (e1, B0 * 128)):
                for k, (tr, e, r) in enumerate(part):
                    j = base + off + k
                    g1r[j] = tr if off == 0 else tr - HALF
                    g1a[j] = tr
                    lenv[j] = el[e]
                    blk, p = j // 128, j % 128
                    s_em[p, blk * 128 + r] = 1
                    s_nm[r, blk * 128 + p] = 1
        cores.append(dict(
            g1i=_wrap16(g1r), g1i_abs=_wrap16(g1a), owni=_wrap16(own),
            len_pl=np.ascontiguousarray(
                lenv.reshape(NT * C, 128).T).astype(np.float32),
            s_em=s_em.astype(BF), s_nm=s_nm.astype(BF), node_of=node_of,
        ))
    return NT, C, cores


# ====================== specialized program (g = 0.5) ======================

def _build_program_special(NT, C):
    _install_act_table_patch()
    B0 = C // 2
    EC = NT * C * 128
    NBLK = NT * C
    G2E = 16 * C          # psS regions
    P1E = G2E
    SCE0 = P1E + 4
    CH0 = SCE0 + 4 * C
    PS_T = CH0 + 8
    nc = bacc.Bacc("TRN2", target_bir_lowering=False, debug=False,
                   num_devices=NCORES)

    xT_d = nc.dram_tensor("xT_in", [65, NPC_PAD], bf16, kind="ExternalInput")
    xpT_d = nc.dram_tensor("xpT_in", [65, NT * 128], bf16, kind="ExternalInput")
    x_perm = nc.dram_tensor("x_perm", [NT * 128, F], f32, kind="ExternalInput")
    g1i_d = nc.dram_tensor("g1i", [128, EC // 16], i16, kind="ExternalInput")
    len_d = nc.dram_tensor("len_pl", [128, NBLK], f32, kind="ExternalInput")
    sem_d = nc.dram_tensor("s_em", [128, EC], bf16, kind="ExternalInput")
    snm_d = nc.dram_tensor("s_nm", [128, EC], bf16, kind="ExternalInput")
    wq_d = nc.dram_tensor("wq", [F, 256], bf16, kind="ExternalInput")
    wuv8_d = nc.dram_tensor("wuv8", [F, 8], bf16, kind="ExternalInput")
    wuvx_d = nc.dram_tensor("wuvx", [65, 16], bf16, kind="ExternalInput")
    dw1_d = nc.dram_tensor("dw1", [F, H * M], bf16, kind="ExternalInput")
    tw1_d = nc.dram_tensor("tw1", [F, H * M], bf16, kind="ExternalInput")
    db1_d = nc.dram_tensor("db1c", [H * M, 1], f32, kind="ExternalInput")
    tb1_d = nc.dram_tensor("tb1c", [H * M, 1], f32, kind="ExternalInput")
    dw2_d = nc.dram_tensor("dw2b", [H * M, 16], bf16, kind="ExternalInput")
    tw2_d = nc.dram_tensor("tw2b", [H * M, 16], bf16, kind="ExternalInput")
    out_d = nc.dram_tensor("out_perm", [NT * 128, F], f32, kind="ExternalOutput")

    scal_d = nc.dram_tensor("scal_d", [NT * 128, 16], f32)
    tbl_shs = [nc.dram_tensor(f"tbl_sh{b}", [AGB, ROWW_S], bf16)
               for b in range(2)]
    tables = [nc.dram_tensor(f"table{b}", [HALF, ROWW_S], bf16,
                             addr_space="Shared") for b in range(2)]

    with tile.TileContext(nc) as tc:
        # ===== Phase A part 1: q|uv table (feeds the AllGathers ASAP) =====
        with (
            tc.tile_pool(name="pa_const", bufs=1) as pc_,
            tc.tile_pool(name="pa", bufs=2) as pa,
            tc.tile_pool(name="pa_ps", bufs=2, space="PSUM") as pap,
        ):
            wq = pc_.tile([F, 256], bf16)
            nc.sync.dma_start(wq[:], wq_d[:])
            wuv8 = pc_.tile([F, 8], bf16)
            nc.sync.dma_start(wuv8[:], wuv8_d[:])
            dw1 = pc_.tile([F, H * M], bf16)
            nc.sync.dma_start(dw1[:], dw1_d[:])
            tw1 = pc_.tile([F, H * M], bf16)
            nc.sync.dma_start(tw1[:], tw1_d[:])
            dw2 = pc_.tile([H * M, 16], bf16)
            nc.sync.dma_start(dw2[:], dw2_d[:])
            tw2 = pc_.tile([H * M, 16], bf16)
            nc.sync.dma_start(tw2[:], tw2_d[:])
            db1 = pc_.tile([H * M, 1], f32)
            nc.sync.dma_start(db1[:], db1_d[:])
            tb1 = pc_.tile([H * M, 1], f32)
            nc.sync.dma_start(tb1[:], tb1_d[:])
            wuvx = pc_.tile([65, 16], bf16)
            nc.sync.dma_start(wuvx[:], wuvx_d[:])

            for b in range(NPC_PAD // 512):
                xT = pa.tile([65, 512], bf16, tag="xT")
                nc.sync.dma_start(xT[:], xT_d[:, b * 512:(b + 1) * 512])
                for it in range(4):
                    r0 = b * 512 + it * 128
                    sl = slice(it * 128, (it + 1) * 128)
                    ps_f = pap.tile([128, 256], f32, tag="ps_f")
                    nc.tensor.matmul(ps_f[:], xT[0:64, sl], wq[:],
                                     start=True, stop=True)
                    ps_u = pap.tile([128, 8], f32, tag="ps_u")
                    nc.tensor.matmul(ps_u[:], xT[0:64, sl], wuv8[:],
                                     start=True, stop=True)
                    rt = pa.tile([128, ROWW_S], bf16, tag="rt")
                    nc.vector.tensor_copy(rt[:, 0:256], ps_f[:])
                    nc.vector.tensor_copy(rt[:, 256:264], ps_u[:])
                    nc.sync.dma_start(
                        tbl_shs[r0 // AGB][r0 % AGB:r0 % AGB + 128, :], rt[:])

            # ===== Phase A part 2: own-node scalars -> local scal_d =====
            for b in range(NT * 128 // 512):
                xpb = pa.tile([65, 512], bf16, tag="xpb")
                nc.sync.dma_start(xpb[:], xpT_d[:, b * 512:(b + 1) * 512])
                ps_h1 = pap.tile([128, 512], f32, tag="ps_h")
                nc.tensor.matmul(ps_h1[:], dw1[:], xpb[0:64, :],
                                 start=True, stop=True)
                h1 = pa.tile([128, 512], bf16, tag="h1")
                nc.scalar.activation(h1[:], ps_h1[:], AF.Silu, bias=db1[:])
                ps_h2 = pap.tile([128, 512], f32, tag="ps_h")
                nc.tensor.matmul(ps_h2[:], tw1[:], xpb[0:64, :],
                                 start=True, stop=True)
                h2 = pa.tile([128, 512], bf16, tag="h2")
                nc.scalar.activation(h2[:], ps_h2[:], AF.Silu, bias=tb1[:])
                for it in range(4):
                    r0 = b * 512 + it * 128
                    sl = slice(it * 128, (it + 1) * 128)
                    ps_s = pap.tile([128, 16], f32, tag="ps_u")
                    nc.tensor.matmul(ps_s[:], xpb[0:65, sl], wuvx[:],
                                     start=True, stop=False,
                                     skip_group_check=True)
                    nc.tensor.matmul(ps_s[:], h1[:, sl], dw2[:],
                                     start=False, stop=False,
                                     skip_group_check=True)
                    nc.tensor.matmul(ps_s[:], h2[:, sl], tw2[:],
                                     start=False, stop=True,
                                     skip_group_check=True)
                    # irt = 1/(softplus(t)+1e-4) replaces the raw t channel
                    itmp = pa.tile([128, 4], f32, tag="itmp")
                    nc.scalar.activation(itmp[:], ps_s[:, 9:16:2], AF.Exp)
                    nc.scalar.activation(itmp[:], itmp[:], AF.Ln, bias=1.0)
                    nc.vector.tensor_scalar_add(itmp[:], itmp[:], 1e-4)
                    nc.vector.reciprocal(itmp[:], itmp[:])
                    so = pa.tile([128, 16], f32, tag="so")
                    nc.vector.tensor_copy(so[:], ps_s[:])
                    nc.vector.tensor_copy(so[:, 9:16:2], itmp[:])
                    nc.sync.dma_start(scal_d[r0:r0 + 128, :], so[:])

        # ===== AllGather both table halves =====
        for b in range(2):
            nc.gpsimd.collective_compute(
                "AllGather", ALU.bypass,
                ins=[tbl_shs[b][:]],
                outs=[tables[b][:]],
                replica_groups=[list(range(NCORES))],
            )

        # ======================= Phase B =======================
        with (
            tc.tile_pool(name="pb_const", bufs=1) as pbc,
            tc.tile_pool(name="pb", bufs=3) as pb,
            tc.tile_pool(name="pbg", bufs=3) as pbg,
            tc.tile_pool(name="pb_val", bufs=2) as pv,
            tc.tile_pool(name="pb_fin", bufs=2) as pf,
            tc.tile_pool(name="ps_main", bufs=2, space="PSUM") as psm,
            tc.tile_pool(name="ps_small", bufs=2, space="PSUM") as pss,
        ):
            g1i = pbc.tile([128, EC // 16], i16)
            nc.sync.dma_start(g1i[:], g1i_d[:])
            len_pl = pbc.tile([128, NBLK], f32)
            nc.sync.dma_start(len_pl[:], len_d[:])
            wq2 = pbc.tile([F, 256], bf16)
            nc.sync.dma_start(wq2[:], wq_d[:])

            for t in range(NT):
                j0 = t * C
                lsl = slice(j0, j0 + C)
                S_t = pb.tile([128, C * 128], bf16, tag="S")
                nc.sync.dma_start(S_t[:], sem_d[:, t * C * 128:(t + 1) * C * 128])
                Sn_t = pb.tile([128, C * 128], bf16, tag="Sn")
                nc.sync.dma_start(Sn_t[:], snm_d[:, t * C * 128:(t + 1) * C * 128])
                g1t = pbg.tile([128, C, ROWW_S], bf16, tag="g1t")
                i0 = t * C * 8
                nc.gpsimd.dma_gather(g1t[:, 0:B0, :], tables[0][:, 0:ROWW_S],
                                     g1i[:, i0:i0 + B0 * 8],
                                     B0 * 128, B0 * 128, elem_size=ROWW_S,
                                     elem_step=ROWW_S, single_packet=False)
                nc.gpsimd.dma_gather(g1t[:, B0:C, :], tables[1][:, 0:ROWW_S],
                                     g1i[:, i0 + B0 * 8:i0 + C * 8],
                                     (C - B0) * 128, (C - B0) * 128,
                                     elem_size=ROWW_S,
                                     elem_step=ROWW_S, single_packet=False)
                # own-node q (no gather needed)
                xoT = pb.tile([65, 128], bf16, tag="xoT")
                nc.sync.dma_start(xoT[:], xpT_d[:, t * 128:(t + 1) * 128])
                ps_qo = pss.tile([128, 256], f32, tag="ps_qo")
                nc.tensor.matmul(ps_qo[:], xoT[0:64, :], wq2[:],
                                 start=True, stop=True)
                qob = pf.tile([128, 256], bf16, tag="qob")
                nc.scalar.activation(qob[:], ps_qo[:], AF.Copy)
                # own-node scalars from local DRAM
                gscf = pf.tile([128, 16], f32, tag="gscf")
                nc.sync.dma_start(gscf[:], scal_d[t * 128:(t + 1) * 128, :])
                gscb = pf.tile([128, 16], bf16, tag="gscb")
                nc.vector.tensor_copy(gscb[:], gscf[:])
                psS = pss.tile([128, PS_T], f32, tag="psS")
                for c in range(C):
                    nc.tensor.matmul(psS[:, c * 16:(c + 1) * 16],
                                     Sn_t[:, c * 128:(c + 1) * 128], gscb[:],
                                     start=True, stop=True,
                                     skip_group_check=True)
                g2v = psS[:, 0:G2E].rearrange("p (c k) -> p c k", c=C)
                sUV = g1t[:, :, 256:264]
                dUV = pf.tile([128, C, 8], f32, tag="dUV")
                nc.vector.tensor_tensor(dUV[:], sUV, g2v[:, :, 0:8],
                                        op=ALU.subtract)
                dU = dUV[:].rearrange("p c (h q) -> p c h q", q=2)[:, :, :, 0]
                dV = dUV[:].rearrange("p c (h q) -> p c h q", q=2)[:, :, :, 1]
                d_r = g2v[:, :, 8:16].rearrange("p c (h q) -> p c h q", q=2)[:, :, :, 0]
                irt_e = g2v[:, :, 8:16].rearrange("p c (h q) -> p c h q", q=2)[:, :, :, 1]
                len_bc = len_pl[:, lsl].unsqueeze(2).broadcast_to([128, C, H])
                LRT = pf.tile([128, C, H], f32, tag="LRT")
                TMPa = pf.tile([128, C, H], f32, tag="TMPa")
                nc.vector.tensor_tensor(TMPa[:], d_r, len_bc, op=ALU.mult)
                nc.vector.tensor_tensor(LRT[:], dU, TMPa[:], op=ALU.subtract)
                nc.vector.tensor_tensor(LRT[:], LRT[:], irt_e, op=ALU.mult)
                P1t = pf.tile([128, C, H], bf16, tag="P1t")
                nc.scalar.activation(P1t[:], LRT[:], AF.Exp, scale=0.5)
                APL = pf.tile([128, C, 8], bf16, tag="APL")
                nc.scalar.activation(APL[:, :, 4:8], dV, AF.Exp)
                for c in range(C):
                    nc.tensor.matmul(psS[:, P1E:P1E + 4],
                                     S_t[:, c * 128:(c + 1) * 128],
                                     P1t[:, c, :],
                                     start=(c == 0), stop=(c == C - 1),
                                     skip_group_check=True)
                sct = pf.tile([128, H], f32, tag="sct")
                nc.vector.tensor_scalar_add(sct[:], psS[:, P1E:P1E + 4], 1e-30)
                nc.vector.reciprocal(sct[:], sct[:])
                scv = pf.tile([128, H], bf16, tag="scv")
                nc.vector.tensor_copy(scv[:], sct[:])
                for c in range(C):
                    nc.tensor.matmul(psS[:, SCE0 + c * 4:SCE0 + c * 4 + 4],
                                     Sn_t[:, c * 128:(c + 1) * 128], scv[:],
                                     start=True, stop=True,
                                     skip_group_check=True)
                sce = psS[:, SCE0:SCE0 + 4 * C].rearrange("p (c k) -> p c k", c=C)
                ert = pf.tile([128, C, H], f32, tag="ert")
                nc.vector.tensor_tensor(ert[:], P1t[:], sce, op=ALU.mult)
                nc.vector.tensor_tensor(APL[:, :, 0:4], ert[:], ert[:],
                                        op=ALU.mult)
                edup = pf.tile([128, C, 8, 2], bf16, tag="edup")
                nc.vector.tensor_copy(
                    edup[:], APL[:].unsqueeze(3).broadcast_to([128, C, 8, 2]))
                val = pv.tile([128, C, 512], bf16, tag="val")
                for a in range(2):
                    for h in range(H):
                        o = a * 256 + h * 64
                        nc.vector.tensor_tensor(
                            val[:, :, o:o + 64].rearrange(
                                "p c (f2 q) -> p c f2 q", q=2),
                            g1t[:, :, h * 64:h * 64 + 64].rearrange(
                                "p c (f2 q) -> p c f2 q", q=2),
                            edup[:, :, a * 4 + h, :].unsqueeze(2)
                                .broadcast_to([128, C, 32, 2]),
                            op=ALU.mult)
                ps_main = psm.tile([128, 512], f32, tag="ps_main")
                for c in range(C):
                    sL = S_t[:, c * 128:(c + 1) * 128]
                    nc.tensor.matmul(ps_main[:], sL, val[:, c, :],
                                     start=(c == 0), stop=(c == C - 1))
                    nc.tensor.matmul(psS[:, CH0:CH0 + 8], sL, APL[:, c, :],
                                     start=(c == 0), stop=(c == C - 1),
                                     skip_group_check=True)
                # finalize: m = 0.25*(iDr*P_er + iDt*P_et) - C1*q_own
                #           C1 = 0.25*(Dr/(Dr+e) + Dt/(Dt+e))
                i8 = pf.tile([128, 8], f32, tag="i8")
                nc.vector.tensor_scalar_add(i8[:], psS[:, CH0:CH0 + 8], 1e-9)
                nc.vector.reciprocal(i8[:], i8[:])
                t8 = pf.tile([128, 8], f32, tag="t8")
                nc.vector.tensor_tensor(t8[:], i8[:], psS[:, CH0:CH0 + 8],
                                        op=ALU.mult)
                i12 = pf.tile([128, 12], f32, tag="i12")
                nc.vector.tensor_scalar_mul(i12[:, 0:8], i8[:], 0.25)
                nc.vector.tensor_tensor(i12[:, 8:12], t8[:, 0:4], t8[:, 4:8],
                                        op=ALU.add)
                nc.vector.tensor_scalar_mul(i12[:, 8:12], i12[:, 8:12], 0.25)
                idup = pf.tile([128, 12, 2], bf16, tag="idup")
                nc.vector.tensor_copy(
                    idup[:], i12[:].unsqueeze(2).broadcast_to([128, 12, 2]))
                Pm = pf.tile([128, 512], bf16, tag="Pm")
                nc.scalar.activation(Pm[:], ps_main[:], AF.Copy)
                tA = pf.tile([128, 256], bf16, tag="tA")
                tB = pf.tile([128, 256], bf16, tag="tB")
                nc.vector.tensor_tensor(
                    tA[:].rearrange("p (h f2 q) -> p h f2 q", h=H, q=2),
                    Pm[:, 0:256].rearrange("p (h f2 q) -> p h f2 q", h=H, q=2),
                    idup[:, 0:4, :].unsqueeze(2)
                        .broadcast_to([128, H, 32, 2]),
                    op=ALU.mult)
                nc.vector.tensor_tensor(
                    tB[:].rearrange("p (h f2 q) -> p h f2 q", h=H, q=2),
                    Pm[:, 256:512].rearrange("p (h f2 q) -> p h f2 q", h=H, q=2),
                    idup[:, 4:8, :].unsqueeze(2)
                        .broadcast_to([128, H, 32, 2]),
                    op=ALU.mult)
                nc.vector.tensor_tensor(tA[:], tA[:], tB[:], op=ALU.add)
                nc.vector.tensor_tensor(
                    tB[:].rearrange("p (h f2 q) -> p h f2 q", h=H, q=2),
                    qob[:].rearrange("p (h f2 q) -> p h f2 q", h=H, q=2),
                    idup[:, 8:12, :].unsqueeze(2)
                        .broadcast_to([128, H, 32, 2]),
                    op=ALU.mult)
                nc.vector.tensor_tensor(tA[:], tA[:], tB[:], op=ALU.subtract)
                m1 = pf.tile([128, 128], bf16, tag="m1")
                nc.vector.tensor_tensor(m1[:], tA[:, 0:128], tA[:, 128:256],
                                        op=ALU.add)
                mmb = pf.tile([128, F], bf16, tag="mmb")
                nc.vector.tensor_tensor(mmb[:], m1[:, 0:64], m1[:, 64:128],
                                        op=ALU.add)
                xp = pb.tile([128, F], f32, tag="xp")
                nc.sync.dma_start(xp[:], x_perm[t * 128:(t + 1) * 128, :])
                ob = pf.tile([128, F], f32, tag="ob")
                nc.vector.tensor_tensor(ob[:], mmb[:], xp[:], op=ALU.add)
                nc.sync.dma_start(out_d[t * 128:(t + 1) * 128, :], ob[:])

    nc.compile()
    return nc


# ====================== general program (any weights) ======================

def _build_program_general(NT, C, w):
    _install_act_table_patch()
    EC = NT * C * 128
    NBLK = NT * C
    G2E = 16 * C
    P1E = G2E
    SCE0 = P1E + 4
    CH0 = SCE0 + 4 * C
    PS_T = CH0 + 24
    nc = bacc.Bacc("TRN2", target_bir_lowering=False, debug=False,
                   num_devices=NCORES)

    xT_d = nc.dram_tensor("xT_in", [65, NPC_PAD], bf16, kind="ExternalInput")
    x_perm = nc.dram_tensor("x_perm", [NT * 128, F], f32, kind="ExternalInput")
    g1i_d = nc.dram_tensor("g1i_abs", [128, EC // 16], i16, kind="ExternalInput")
    owni_d = nc.dram_tensor("owni", [128, NT * 8], i16, kind="ExternalInput")
    len_d = nc.dram_tensor("len_pl", [128, NBLK], f32, kind="ExternalInput")
    sem_d = nc.dram_tensor("s_em", [128, EC], bf16, kind="ExternalInput")
    snm_d = nc.dram_tensor("s_nm", [128, EC], bf16, kind="ExternalInput")
    wrwt_d = nc.dram_tensor("wrwt", [F, 512], bf16, kind="ExternalInput")
    wuv_d = nc.dram_tensor("wuv_ext", [65, 16], bf16, kind="ExternalInput")
    dw1_d = nc.dram_tensor("dw1", [F, H * M], bf16, kind="ExternalInput")
    tw1_d = nc.dram_tensor("tw1", [F, H * M], bf16, kind="ExternalInput")
    db1_d = nc.dram_tensor("db1c", [H * M, 1], f32, kind="ExternalInput")
    tb1_d = nc.dram_tensor("tb1c", [H * M, 1], f32, kind="ExternalInput")
    dw2_d = nc.dram_tensor("dw2b", [H * M, 16], bf16, kind="ExternalInput")
    tw2_d = nc.dram_tensor("tw2b", [H * M, 16], bf16, kind="ExternalInput")
    rtw_d = nc.dram_tensor("rtw_sb", [128, H], f32, kind="ExternalInput")
    out_d = nc.dram_tensor("out_perm", [NT * 128, F], f32, kind="ExternalOutput")

    table_shs = [nc.dram_tensor(f"table_sh{b}", [AGB, ROWW_G], bf16)
                 for b in range(NPC_PAD // AGB)]
    table = nc.dram_tensor("table", [TROW_TOT, ROWW_G], bf16, addr_space="Shared")

    ms = w["mix_scale"]
    mb = w["mix_bias"]

    with tile.TileContext(nc) as tc:
        with (
            tc.tile_pool(name="pa_const", bufs=1) as pc_,
            tc.tile_pool(name="pa", bufs=2) as pa,
            tc.tile_pool(name="pa_ps", bufs=2, space="PSUM") as pap,
        ):
            wrwt = pc_.tile([F, 512], bf16)
            nc.sync.dma_start(wrwt[:], wrwt_d[:])
            wuv = pc_.tile([65, 16], bf16)
            nc.sync.dma_start(wuv[:], wuv_d[:])
            dw1 = pc_.tile([F, H * M], bf16)
            nc.sync.dma_start(dw1[:], dw1_d[:])
            tw1 = pc_.tile([F, H * M], bf16)
            nc.sync.dma_start(tw1[:], tw1_d[:])
            dw2 = pc_.tile([H * M, 16], bf16)
            nc.sync.dma_start(dw2[:], dw2_d[:])
            tw2 = pc_.tile([H * M, 16], bf16)
            nc.sync.dma_start(tw2[:], tw2_d[:])
            db1 = pc_.tile([H * M, 1], f32)
            nc.sync.dma_start(db1[:], db1_d[:])
            tb1 = pc_.tile([H * M, 1], f32)
            nc.sync.dma_start(tb1[:], tb1_d[:])

            for b in range(NPC_PAD // 512):
                xT = pa.tile([65, 512], bf16, tag="xT")
                nc.sync.dma_start(xT[:], xT_d[:, b * 512:(b + 1) * 512])
                ps_h1 = pap.tile([128, 512], f32, tag="ps_h")
                nc.tensor.matmul(ps_h1[:], dw1[:], xT[0:64, :],
                                 start=True, stop=True)
                h1 = pa.tile([128, 512], bf16, tag="h1")
                nc.scalar.activation(h1[:], ps_h1[:], AF.Silu, bias=db1[:])
                ps_h2 = pap.tile([128, 512], f32, tag="ps_h")
                nc.tensor.matmul(ps_h2[:], tw1[:], xT[0:64, :],
                                 start=True, stop=True)
                h2 = pa.tile([128, 512], bf16, tag="h2")
                nc.scalar.activation(h2[:], ps_h2[:], AF.Silu, bias=tb1[:])
                for it in range(4):
                    r0 = b * 512 + it * 128
                    sl = slice(it * 128, (it + 1) * 128)
                    ps_f = pap.tile([128, 512], f32, tag="ps_f")
                    nc.tensor.matmul(ps_f[:], xT[0:64, sl], wrwt[:],
                                     start=True, stop=True)
                    ps_s = pap.tile([128, 16], f32, tag="ps_s")
                    nc.tensor.matmul(ps_s[:], xT[0:65, sl], wuv[:],
                                     start=True, stop=False,
                                     skip_group_check=True)
                    nc.tensor.matmul(ps_s[:], h1[:, sl], dw2[:],
                                     start=False, stop=False,
                                     skip_group_check=True)
                    nc.tensor.matmul(ps_s[:], h2[:, sl], tw2[:],
                                     start=False, stop=True,
                                     skip_group_check=True)
                    rt = pa.tile([128, ROWW_G], bf16, tag="rt")
                    nc.vector.tensor_copy(rt[:, 0:512], ps_f[:])
                    nc.vector.tensor_copy(rt[:, 512:544].bitcast(f32), ps_s[:])
                    nc.sync.dma_start(
                        table_shs[r0 // AGB][r0 % AGB:r0 % AGB + 128, :], rt[:])

        for b in range(NPC_PAD // AGB):
            nc.gpsimd.collective_compute(
                "AllGather", ALU.bypass,
                ins=[table_shs[b][:]],
                outs=[table[b * AGB * NCORES:(b + 1) * AGB * NCORES, :]],
                replica_groups=[list(range(NCORES))],
            )

        with (
            tc.tile_pool(name="pb_const", bufs=1) as pbc,
            tc.tile_pool(name="pb_planes", bufs=1) as ppl,
            tc.tile_pool(name="pb", bufs=3) as pb,
            tc.tile_pool(name="pbg", bufs=3) as pbg,
            tc.tile_pool(name="pb_val", bufs=2) as pv,
            tc.tile_pool(name="pb_fin", bufs=2) as pf,
            tc.tile_pool(name="ps_main", bufs=2, space="PSUM") as psm,
            tc.tile_pool(name="ps_small", bufs=2, space="PSUM") as pss,
        ):
            g1i = pbc.tile([128, EC // 16], i16)
            nc.sync.dma_start(g1i[:], g1i_d[:])
            owni = pbc.tile([128, NT * 8], i16)
            nc.sync.dma_start(owni[:], owni_d[:])
            len_pl = pbc.tile([128, NBLK], f32)
            nc.sync.dma_start(len_pl[:], len_d[:])
            rtw_sb = pbc.tile([128, H], f32)
            nc.sync.dma_start(rtw_sb[:], rtw_d[:])

            LEN4 = ppl.tile([128, NBLK, H], f32)
            Q1 = ppl.tile([128, NBLK, H], f32)
            Q2 = ppl.tile([128, NBLK, H], f32)
            Q3 = ppl.tile([128, NBLK, H], f32)
            len_b = len_pl[:].unsqueeze(2).broadcast_to([128, NBLK, H])
            rtw_b = rtw_sb[:].unsqueeze(1).broadcast_to([128, NBLK, H])
            nc.vector.tensor_tensor(LEN4[:], len_b, rtw_b, op=ALU.mult)
            G = ppl.tile([128, NBLK, H], f32)
            for h in range(H):
                nc.scalar.activation(G[:, :, h:h + 1],
                                     len_pl[:].unsqueeze(2), AF.Sigmoid,
                                     scale=float(ms[h]), bias=float(mb[h]))
            OMG = ppl.tile([128, NBLK, H], f32)
            nc.scalar.activation(OMG[:], G[:], AF.Copy, scale=-1.0, bias=1.0)
            nc.vector.tensor_tensor(Q1[:], G[:], G[:], op=ALU.mult)
            nc.vector.tensor_tensor(Q2[:], G[:], OMG[:], op=ALU.mult)
            nc.vector.tensor_tensor(Q3[:], OMG[:], OMG[:], op=ALU.mult)

            for t in range(NT):
                j0 = t * C
                lsl = slice(j0, j0 + C)
                S_t = pb.tile([128, C * 128], bf16, tag="S")
                nc.sync.dma_start(S_t[:], sem_d[:, t * C * 128:(t + 1) * C * 128])
                Sn_t = pb.tile([128, C * 128], bf16, tag="Sn")
                nc.sync.dma_start(Sn_t[:], snm_d[:, t * C * 128:(t + 1) * C * 128])
                g1t = pbg.tile([128, C, ROWW_G], bf16, tag="g1t")
                nc.gpsimd.dma_gather(g1t[:], table[:, 0:ROWW_G],
                                     g1i[:, t * C * 8:(t + 1) * C * 8],
                                     C * 128, C * 128, elem_size=ROWW_G,
                                     elem_step=ROWW_G, single_packet=False)
                gown = pbg.tile([128, 1, ROWW_G], bf16, tag="gown")
                nc.gpsimd.dma_gather(gown[:], table[:, 0:ROWW_G],
                                     owni[:, t * 8:(t + 1) * 8],
                                     128, 128, elem_size=ROWW_G,
                                     elem_step=ROWW_G, single_packet=False)
                psS = pss.tile([128, PS_T], f32, tag="psS")
                gsc = pf.tile([128, 16], bf16, tag="gsc")
                nc.vector.tensor_copy(gsc[:], gown[:, 0, 512:544].bitcast(f32))
                for c in range(C):
                    nc.tensor.matmul(psS[:, c * 16:(c + 1) * 16],
                                     Sn_t[:, c * 128:(c + 1) * 128], gsc[:],
                                     start=True, stop=True,
                                     skip_group_check=True)
                g2v = psS[:, 0:G2E].rearrange("p (c k) -> p c k", c=C)
                sUV = g1t[:, :, 512:544].bitcast(f32)
                dUV = pf.tile([128, C, 8], f32, tag="dUV")
                nc.vector.tensor_tensor(dUV[:], sUV[:, :, 0:8], g2v[:, :, 0:8],
                                        op=ALU.subtract)
                dU = dUV[:].rearrange("p c (h q) -> p c h q", q=2)[:, :, :, 0]
                dV = dUV[:].rearrange("p c (h q) -> p c h q", q=2)[:, :, :, 1]
                d_r = g2v[:, :, 8:16].rearrange("p c (h q) -> p c h q", q=2)[:, :, :, 0]
                t_r = g2v[:, :, 8:16].rearrange("p c (h q) -> p c h q", q=2)[:, :, :, 1]
                len_bc = len_pl[:, lsl].unsqueeze(2).broadcast_to([128, C, H])
                LRT = pf.tile([128, C, H], f32, tag="LRT")
                TMPa = pf.tile([128, C, H], f32, tag="TMPa")
                nc.vector.tensor_tensor(TMPa[:], d_r, len_bc, op=ALU.mult)
                nc.vector.tensor_tensor(LRT[:], dU, TMPa[:], op=ALU.subtract)
                nc.vector.tensor_tensor(TMPa[:], LEN4[:, lsl, :], t_r, op=ALU.add)
                nc.scalar.activation(TMPa[:], TMPa[:], AF.Exp)
                nc.scalar.activation(TMPa[:], TMPa[:], AF.Ln, bias=1.0)
                nc.vector.tensor_scalar_add(TMPa[:], TMPa[:], 1e-4)
                nc.vector.reciprocal(TMPa[:], TMPa[:])
                nc.vector.tensor_tensor(LRT[:], LRT[:], TMPa[:], op=ALU.mult)
                P1t = pf.tile([128, C, H], bf16, tag="P1t")
                nc.scalar.activation(P1t[:], LRT[:], AF.Exp, scale=0.5)
                APL = pf.tile([128, C, 24], bf16, tag="APL")
                nc.scalar.activation(APL[:, :, 20:24], dV, AF.Exp)
                for c in range(C):
                    nc.tensor.matmul(psS[:, P1E:P1E + 4],
                                     S_t[:, c * 128:(c + 1) * 128],
                                     P1t[:, c, :],
                                     start=(c == 0), stop=(c == C - 1),
                                     skip_group_check=True)
                scv = pf.tile([128, H], bf16, tag="scv")
                sct = pf.tile([128, H], f32, tag="sct")
                nc.vector.tensor_scalar_add(sct[:], psS[:, P1E:P1E + 4], 1e-30)
                nc.vector.reciprocal(sct[:], sct[:])
                nc.vector.tensor_copy(scv[:], sct[:])
                for c in range(C):
                    nc.tensor.matmul(psS[:, SCE0 + c * 4:SCE0 + c * 4 + 4],
                                     Sn_t[:, c * 128:(c + 1) * 128], scv[:],
                                     start=True, stop=True,
                                     skip_group_check=True)
                sce = psS[:, SCE0:SCE0 + 4 * C].rearrange("p (c k) -> p c k", c=C)
                ert = pf.tile([128, C, H], f32, tag="ert")
                nc.vector.tensor_tensor(ert[:], P1t[:], sce, op=ALU.mult)
                nc.vector.tensor_tensor(APL[:, :, 16:20], ert[:], ert[:],
                                        op=ALU.mult)
                nc.vector.tensor_tensor(APL[:, :, 0:4], Q1[:, lsl, :],
                                        APL[:, :, 16:20], op=ALU.mult)
                nc.vector.tensor_tensor(APL[:, :, 4:8], Q2[:, lsl, :],
                                        APL[:, :, 20:24], op=ALU.mult)
                nc.vector.tensor_tensor(APL[:, :, 8:12], Q2[:, lsl, :],
                                        APL[:, :, 16:20], op=ALU.mult)
                nc.vector.tensor_tensor(APL[:, :, 12:16], Q3[:, lsl, :],
                                        APL[:, :, 20:24], op=ALU.mult)
                wdup = pf.tile([128, C, 16, 2], bf16, tag="wdup")
                nc.vector.tensor_copy(
                    wdup[:], APL[:, :, 0:16].unsqueeze(3)
                        .broadcast_to([128, C, 16, 2]))
                val1 = pv.tile([128, C, 512], bf16, tag="val1")
                val2 = pv.tile([128, C, 512], bf16, tag="val2")
                for (vt, fs0, wi) in ((val1, 0, 0), (val1, 256, 4),
                                      (val2, 0, 8), (val2, 256, 12)):
                    src0 = 0 if wi < 8 else 256
                    for h in range(H):
                        nc.vector.tensor_tensor(
                            vt[:, :, fs0 + h * 64:fs0 + h * 64 + 64].rearrange(
                                "p c (f2 q) -> p c f2 q", q=2),
                            g1t[:, :, src0 + h * 64:src0 + h * 64 + 64].rearrange(
                                "p c (f2 q) -> p c f2 q", q=2),
                            wdup[:, :, wi + h, :].unsqueeze(2)
                                .broadcast_to([128, C, 32, 2]),
                            op=ALU.mult)
                ps_main = psm.tile([128, 512], f32, tag="ps_main")
                for c in range(C):
                    sL = S_t[:, c * 128:(c + 1) * 128]
                    nc.tensor.matmul(ps_main[:], sL, val1[:, c, :],
                                     start=(c == 0), stop=False)
                    nc.tensor.matmul(ps_main[:], sL, val2[:, c, :],
                                     start=False, stop=(c == C - 1))
                    nc.tensor.matmul(psS[:, CH0:CH0 + 24], sL, APL[:, c, :],
                                     start=(c == 0), stop=(c == C - 1),
                                     skip_group_check=True)
                ch = psS[:, CH0:CH0 + 24]
                idrc = pf.tile([128, 16], f32, tag="idrc")
                nc.vector.tensor_scalar_add(idrc[:, 0:8], ch[:, 16:24], 1e-9)
                nc.vector.reciprocal(idrc[:, 0:8], idrc[:, 0:8])
                tmp16 = pf.tile([128, 2, 8], f32, tag="tmp16")
                nc.vector.tensor_tensor(
                    tmp16[:],
                    idrc[:, 0:8].unsqueeze(1).broadcast_to([128, 2, 8]),
                    ch[:, 0:16].rearrange("p (a k) -> p a k", a=2),
                    op=ALU.mult)
                t16 = tmp16[:].rearrange("p a k -> p (a k)")
                nc.vector.tensor_tensor(
                    idrc[:, 8:16].rearrange("p (a h) -> p a h", a=2),
                    t16.rearrange("p (a g h) -> p a g h", a=2, g=2)[:, :, 0, :],
                    t16.rearrange("p (a g h) -> p a g h", a=2, g=2)[:, :, 1, :],
                    op=ALU.add)
                idup = pf.tile([128, 16, 2], bf16, tag="idup")
                nc.vector.tensor_copy(
                    idup[:], idrc[:].unsqueeze(2).broadcast_to([128, 16, 2]))
                Pm = pf.tile([128, 512], bf16, tag="Pm")
                nc.scalar.activation(Pm[:], ps_main[:], AF.Copy)
                t1 = pf.tile([128, 512], bf16, tag="t1")
                t2 = pf.tile([128, 512], bf16, tag="t2")
                for a in range(2):
                    asl = slice(a * 256, (a + 1) * 256)
                    nc.vector.tensor_tensor(
                        t1[:, asl].rearrange("p (h f2 q) -> p h f2 q",
                                             h=H, q=2),
                        Pm[:, asl].rearrange("p (h f2 q) -> p h f2 q",
                                             h=H, q=2),
                        idup[:, a * 4:a * 4 + 4, :].unsqueeze(2)
                            .broadcast_to([128, H, 32, 2]),
                        op=ALU.mult)
                    nc.vector.tensor_tensor(
                        t2[:, asl].rearrange("p (h f2 q) -> p h f2 q",
                                             h=H, q=2),
                        gown[:, 0, asl].rearrange("p (h f2 q) -> p h f2 q",
                                                  h=H, q=2),
                        idup[:, 8 + a * 4:8 + a * 4 + 4, :].unsqueeze(2)
                            .broadcast_to([128, H, 32, 2]),
                        op=ALU.mult)
                nc.vector.tensor_tensor(t1[:], t1[:], t2[:], op=ALU.subtract)
                m2 = pf.tile([128, 256], bf16, tag="m2")
                nc.vector.tensor_tensor(m2[:], t1[:, 0:256], t1[:, 256:512],
                                        op=ALU.add)
                m1 = pf.tile([128, 128], bf16, tag="m1")
                nc.vector.tensor_tensor(m1[:], m2[:, 0:128], m2[:, 128:256],
                                        op=ALU.add)
                mmb = pf.tile([128, F], bf16, tag="mmb")
                nc.vector.tensor_tensor(mmb[:], m1[:, 0:64], m1[:, 64:128],
                                        op=ALU.add)
                xp = pb.tile([128, F], f32, tag="xp")
                nc.sync.dma_start(xp[:], x_perm[t * 128:(t + 1) * 128, :])
                ob = pf.tile([128, F], f32, tag="ob")
                nc.vector.tensor_tensor(ob[:], mmb[:], xp[:], op=ALU.add)
                nc.sync.dma_start(out_d[t * 128:(t + 1) * 128, :], ob[:])

    nc.compile()
    return nc


def kernel(**inputs):
    x = np.asarray(inputs["x"], np.float32)
    edge_index = np.asarray(inputs["edge_index"])
    edge_len = np.asarray(inputs["edge_len"], np.float32)

    NT, C, cores = _preprocess(x, edge_index, edge_len)

    Wp = np.asarray(inputs["Wp"], np.float32)
    Wr = np.asarray(inputs["Wr"], np.float32)
    Wt = np.asarray(inputs["Wt"], np.float32)
    Wout = np.asarray(inputs["Wout"], np.float32)
    ms = np.asarray(inputs["mix_scale"], np.float32)
    mb = np.asarray(inputs["mix_bias"], np.float32)
    rtw = np.asarray(inputs["rtw"], np.float32)
    dconst = (np.asarray(inputs["decay_b2"], np.float64)
              + _np_softplus(inputs["rdls"])).astype(np.float32)
    tconst = (np.asarray(inputs["temp_b2"], np.float64)
              + np.asarray(inputs["rtb"], np.float64)).astype(np.float32)

    special = bool(np.all(ms == 0) and np.all(mb == 0) and np.all(rtw == 0))

    rs = np.asarray(inputs["radial_score"], np.float32)
    ts_ = np.asarray(inputs["tangential_score"], np.float32)
    wu = np.einsum("hfg,hg->fh", Wp, rs)                  # [F, H]
    wv = np.einsum("hfg,hg->fh", Wp, ts_)
    wout_f = 0.25 * Wout
    wd1f = np.einsum("hfg,hgm->fhm", Wp, np.asarray(inputs["decay_W1"], np.float32))
    wt1f = np.einsum("hfg,hgm->fhm", Wp, np.asarray(inputs["temp_W1"], np.float32))

    wuv_ext = np.zeros((65, 16), np.float32)
    wuv_ext[0:64, 0:8:2] = wu
    wuv_ext[0:64, 1:8:2] = wv
    wuv_ext[64, 8:16:2] = dconst
    wuv_ext[64, 9:16:2] = tconst
    dw2b = np.zeros((H * M, 16), np.float32)
    tw2b = np.zeros((H * M, 16), np.float32)
    d_w2 = np.asarray(inputs["decay_w2"], np.float32)
    t_w2 = np.asarray(inputs["temp_w2"], np.float32)
    for h in range(H):
        dw2b[h * M:(h + 1) * M, 8 + 2 * h] = d_w2[h]
        tw2b[h * M:(h + 1) * M, 9 + 2 * h] = t_w2[h]

    shared = {
        "dw1": np.ascontiguousarray(wd1f.reshape(F, H * M)).astype(BF),
        "tw1": np.ascontiguousarray(wt1f.reshape(F, H * M)).astype(BF),
        "db1c": np.ascontiguousarray(
            np.asarray(inputs["decay_b1"], np.float32).reshape(H * M, 1)),
        "tb1c": np.ascontiguousarray(
            np.asarray(inputs["temp_b1"], np.float32).reshape(H * M, 1)),
        "dw2b": dw2b.astype(BF),
        "tw2b": tw2b.astype(BF),
    }
    if special:
        wqF = np.einsum("hfg,gk->hfk", Wr + Wt, wout_f)   # [H, F, F]
        shared["wq"] = np.ascontiguousarray(
            wqF.transpose(1, 0, 2).reshape(F, H * F)).astype(BF)
        wuv8 = np.zeros((F, 8), np.float32)
        wuv8[:, 0:8:2] = wu
        wuv8[:, 1:8:2] = wv
        shared["wuv8"] = wuv8.astype(BF)
        shared["wuvx"] = wuv_ext.astype(BF)
        key = ("v3s", NT, C)
        if key not in _CACHE:
            _CACHE[key] = _build_program_special(NT, C)
    else:
        wrF = np.einsum("hfg,gk->hfk", Wr, wout_f)
        wtF = np.einsum("hfg,gk->hfk", Wt, wout_f)
        shared["wrwt"] = np.ascontiguousarray(np.concatenate(
            [wrF.transpose(1, 0, 2).reshape(F, H * F),
             wtF.transpose(1, 0, 2).reshape(F, H * F)], axis=1)).astype(BF)
        shared["wuv_ext"] = wuv_ext.astype(BF)
        shared["rtw_sb"] = np.tile(rtw[None, :], (128, 1))
        key = ("v3g", NT, C, ms.tobytes(), mb.tobytes())
        if key not in _CACHE:
            _CACHE[key] = _build_program_general(
                NT, C, dict(mix_scale=ms, mix_bias=mb))
    nc = _CACHE[key]

    in_maps = []
    for c in range(NCORES):
        cc = cores[c]
        xr = np.zeros((65, NPC_PAD), np.float32)
        xr[0:64, :NPC] = x[c * NPC:(c + 1) * NPC].T
        xr[64, :] = 1.0
        xp = np.zeros((NT * 128, F), np.float32)
        valid = cc["node_of"] >= 0
        xp[valid] = x[cc["node_of"][valid]]
        im = dict(shared, xT_in=xr.astype(BF), x_perm=xp,
                  len_pl=cc["len_pl"], s_em=cc["s_em"], s_nm=cc["s_nm"])
        if special:
            xpt = np.zeros((65, NT * 128), np.float32)
            xpt[0:64, :] = xp.T
            xpt[64, :] = 1.0
            im["xpT_in"] = xpt.astype(BF)
            im["g1i"] = cc["g1i"]
        else:
            im["g1i_abs"] = cc["g1i_abs"]
            im["owni"] = cc["owni"]
        in_maps.append(im)

    r = run_bass_kernel_spmd(nc, in_maps, list(range(NCORES)),
                             trace=TRACE, **TRACE_KW)
    if TRACE:
        LAST_RESULT["exec_time_ns"] = r.exec_time_ns
        LAST_RESULT["mean_exec_time_ns"] = r.mean_exec_time_ns
        LAST_RESULT["raw"] = r

    out = np.array(x, np.float32, copy=True)  # zero-degree nodes: out = x
    for c in range(NCORES):
        cc = cores[c]
        rows = r.results[c]["out_perm"]
        valid = cc["node_of"] >= 0
        out[cc["node_of"][valid]] = rows[valid]
    return out
